# revision 1
# baseline (speedup 1.0000x reference)
"""Trainium2 Bass kernel for LuluAttention (gated GQA attention + RoPE).

Sharding over 8 NeuronCores: core = b*4 + g where b = batch (2), g = head
group (4). Each core computes 4 Q heads + their shared KV head for one batch
element, plus the matching gate slice, and a partial o_proj output
(contraction over its 512 attn dims). Host sums the 4 partials per batch.

All on-chip tensors are kept in transposed layout ([dim, seq]) so the
attention pipeline needs no on-chip transposes:
  qT/kT [d, s]  -> scoresT[sk, sq] = kT_tile.T @ qT_chunk
  softmax over sk (partition dim): denominator via ones-matmul, broadcast of
  the reciprocal via a K=1 matmul.
  v kept straight [s, d] -> attnT[d, sq] = v_tile.T @ probsT
  agT[d, sq] = attnT * gateT * recip  feeds o_proj directly as lhsT.
RoPE rotate-half needs a cross-partition rotation by 64: done with two DMA
copies, signs folded into the host-precomputed sin table.
"""

import numpy as np
import ml_dtypes
from contextlib import ExitStack

import concourse.bass as bass
import concourse.bacc as bacc
import concourse.tile as tile
from concourse import mybir
from concourse.bass_utils import run_bass_kernel_spmd

BF16 = ml_dtypes.bfloat16

HIDDEN = 2048
B = 2
S_FULL = 2048
P = 128
CH = 512               # seq chunk width
QH = 4                 # q heads per core
DQ = QH * P            # 512 q dims per core
KT = HIDDEN // P       # 16 contraction tiles
SCALE = 1.0 / float(np.sqrt(128.0))
ROPE_THETA = 10000.0


def build_program(S=S_FULL):
    f32 = mybir.dt.float32
    bf16 = mybir.dt.bfloat16
    sig = mybir.ActivationFunctionType.Sigmoid
    expf = mybir.ActivationFunctionType.Exp

    NCH = S // CH
    ST = CH // P           # 4 seq sub-tiles per chunk

    nc = bacc.Bacc("TRN2", debug=False, target_bir_lowering=False)

    xT = nc.declare_dram_parameter("xT", [HIDDEN, S], bf16, False)
    wq = nc.declare_dram_parameter("wq", [HIDDEN, DQ], bf16, False)
    wk = nc.declare_dram_parameter("wk", [HIDDEN, P], bf16, False)
    wv = nc.declare_dram_parameter("wv", [HIDDEN, P], bf16, False)
    wg = nc.declare_dram_parameter("wg", [HIDDEN, DQ], bf16, False)
    wo = nc.declare_dram_parameter("wo", [DQ, HIDDEN], bf16, False)
    bg = nc.declare_dram_parameter("bg", [DQ], f32, False)
    cosT = nc.declare_dram_parameter("cosT", [P, S], f32, False)
    sinT = nc.declare_dram_parameter("sinT", [P, S], f32, False)
    msk = nc.declare_dram_parameter("msk", [ST, P, CH], bf16, False)
    out = nc.declare_dram_parameter("out", [S, HIDDEN], f32, True)

    with tile.TileContext(nc) as tc, ExitStack() as ctx:
        wpool = ctx.enter_context(tc.tile_pool(name="weights", bufs=1))
        xpool = ctx.enter_context(tc.tile_pool(name="xchunks", bufs=2))
        qkv = ctx.enter_context(tc.tile_pool(name="qkv", bufs=1))
        work = ctx.enter_context(tc.tile_pool(name="work", bufs=3))
        agp = ctx.enter_context(tc.tile_pool(name="agp", bufs=2))
        outp = ctx.enter_context(tc.tile_pool(name="outp", bufs=2))
        ps_mm = ctx.enter_context(tc.tile_pool(name="ps_mm", bufs=2, space="PSUM"))
        ps_sc = ctx.enter_context(tc.tile_pool(name="ps_sc", bufs=2, space="PSUM"))
        ps_at = ctx.enter_context(tc.tile_pool(name="ps_at", bufs=2, space="PSUM"))
        ps_sm = ctx.enter_context(tc.tile_pool(name="ps_sm", bufs=1, space="PSUM"))

        # ---- persistent loads ----
        wq_sb = wpool.tile([P, KT, DQ], bf16, tag="wq")
        nc.sync.dma_start(out=wq_sb, in_=wq[:, :].rearrange("(kt p) n -> p kt n", p=P))
        wk_sb = wpool.tile([P, KT, P], bf16, tag="wk")
        nc.sync.dma_start(out=wk_sb, in_=wk[:, :].rearrange("(kt p) n -> p kt n", p=P))
        wv_sb = wpool.tile([P, KT, P], bf16, tag="wv")
        nc.sync.dma_start(out=wv_sb, in_=wv[:, :].rearrange("(kt p) n -> p kt n", p=P))
        wg_sb = wpool.tile([P, KT, DQ], bf16, tag="wg")
        nc.sync.dma_start(out=wg_sb, in_=wg[:, :].rearrange("(kt p) n -> p kt n", p=P))
        wo_sb = wpool.tile([P, QH, HIDDEN], bf16, tag="wo")
        nc.sync.dma_start(out=wo_sb, in_=wo[:, :].rearrange("(dt p) n -> p dt n", p=P))
        bg_sb = wpool.tile([P, QH], f32, tag="bg")
        nc.sync.dma_start(out=bg_sb, in_=bg[:].rearrange("(h p) -> p h", p=P))
        cos_sb = wpool.tile([P, S], f32, tag="cos")
        nc.sync.dma_start(out=cos_sb, in_=cosT[:, :])
        sin_sb = wpool.tile([P, S], f32, tag="sin")
        nc.sync.dma_start(out=sin_sb, in_=sinT[:, :])
        msk_sb = wpool.tile([P, ST, CH], bf16, tag="msk")
        nc.sync.dma_start(out=msk_sb, in_=msk[:, :, :].rearrange("o p n -> p o n"))
        ones_pv = wpool.tile([P, 1], bf16, tag="ones_pv")
        nc.vector.memset(ones_pv, 1.0)
        ones_bc = wpool.tile([1, P], f32, tag="ones_bc")
        nc.vector.memset(ones_bc, 1.0)

        # persistent per-core activations (transposed layouts)
        qro = qkv.tile([P, QH, S], bf16, tag="qro")
        kro = qkv.tile([P, S], bf16, tag="kro")
        v_sb = qkv.tile([P, S // P, P], bf16, tag="v")
        gt = qkv.tile([P, QH, S], bf16, tag="gt")

        for c in range(NCH):
            cs = slice(c * CH, (c + 1) * CH)

            # ---- projections for this seq chunk ----
            xc = xpool.tile([P, KT, CH], bf16, tag="xc")
            nc.sync.dma_start(
                out=xc, in_=xT[:, cs].rearrange("(kt p) n -> p kt n", p=P)
            )

            # q heads + k, with RoPE applied out of PSUM
            for qh in range(QH + 1):
                ps = ps_mm.tile([P, CH], f32, tag="proj")
                for kt in range(KT):
                    lhs = (
                        wq_sb[:, kt, qh * P:(qh + 1) * P]
                        if qh < QH
                        else wk_sb[:, kt, :]
                    )
                    nc.tensor.matmul(
                        ps, lhs, xc[:, kt, :], start=(kt == 0), stop=(kt == KT - 1)
                    )
                qf = work.tile([P, CH], f32, tag="qf")
                nc.scalar.copy(out=qf, in_=ps)
                rot = work.tile([P, CH], f32, tag="rot")
                nc.sync.dma_start(out=rot[0:64, :], in_=qf[64:128, :])
                nc.sync.dma_start(out=rot[64:128, :], in_=qf[0:64, :])
                t1 = work.tile([P, CH], f32, tag="t1")
                nc.vector.tensor_mul(t1, qf, cos_sb[:, cs])
                t2 = work.tile([P, CH], f32, tag="t2")
                nc.vector.tensor_mul(t2, rot, sin_sb[:, cs])
                dst = qro[:, qh, cs] if qh < QH else kro[:, cs]
                nc.vector.tensor_add(dst, t1, t2)

            # gate heads: sigmoid(x @ Wg + bg), transposed layout
            for qh in range(QH):
                ps = ps_mm.tile([P, CH], f32, tag="proj")
                for kt in range(KT):
                    nc.tensor.matmul(
                        ps,
                        wg_sb[:, kt, qh * P:(qh + 1) * P],
                        xc[:, kt, :],
                        start=(kt == 0),
                        stop=(kt == KT - 1),
                    )
                nc.scalar.activation(
                    out=gt[:, qh, cs],
                    in_=ps,
                    func=sig,
                    bias=bg_sb[:, qh:qh + 1],
                    scale=1.0,
                )

            # v in straight layout [s, d]
            for st in range(ST):
                s0 = c * ST + st
                ps = ps_mm.tile([P, P], f32, tag="proj")
                for kt in range(KT):
                    nc.tensor.matmul(
                        ps,
                        xc[:, kt, st * P:(st + 1) * P],
                        wv_sb[:, kt, :],
                        start=(kt == 0),
                        stop=(kt == KT - 1),
                    )
                nc.scalar.copy(out=v_sb[:, s0, :], in_=ps)

            # ---- attention for this sq chunk ----
            ag = agp.tile([P, QH, CH], bf16, tag="ag")
            ntiles = (c + 1) * ST
            for qh in range(QH):
                at = ps_at.tile([P, CH], f32, tag="attn")
                dn = ps_sm.tile([1, CH], f32, tag="denom")
                for t in range(ntiles):
                    sc_ps = ps_sc.tile([P, CH], f32, tag="sc")
                    nc.tensor.matmul(
                        sc_ps,
                        kro[:, t * P:(t + 1) * P],
                        qro[:, qh, cs],
                        start=True,
                        stop=True,
                    )
                    pr = work.tile([P, CH], bf16, tag="probs")
                    nc.scalar.activation(out=pr, in_=sc_ps, func=expf, scale=SCALE)
                    o = t - c * ST
                    if o >= 0:
                        nc.vector.tensor_mul(pr, pr, msk_sb[:, o, :])
                    nc.tensor.matmul(
                        at, v_sb[:, t, :], pr,
                        start=(t == 0), stop=(t == ntiles - 1),
                    )
                    nc.tensor.matmul(
                        dn, ones_pv, pr,
                        start=(t == 0), stop=(t == ntiles - 1),
                    )
                rc = work.tile([1, CH], f32, tag="recip")
                nc.vector.reciprocal(rc, dn)
                bc = ps_sm.tile([P, CH], f32, tag="bcast")
                nc.tensor.matmul(bc, ones_bc, rc, start=True, stop=True)
                t3 = work.tile([P, CH], f32, tag="t3")
                nc.vector.tensor_mul(t3, at, gt[:, qh, cs])
                nc.vector.tensor_mul(ag[:, qh, :], t3, bc)

            # ---- partial o_proj for this chunk ----
            for st in range(ST):
                r0 = c * CH + st * P
                for hp in range(HIDDEN // CH // 2):
                    pss = [
                        ps_mm.tile([P, CH], f32, tag="proj", name=f"ops{hi}")
                        for hi in range(2)
                    ]
                    for dt in range(QH):
                        for hi in range(2):
                            h0 = hp * 2 + hi
                            nc.tensor.matmul(
                                pss[hi],
                                ag[:, dt, st * P:(st + 1) * P],
                                wo_sb[:, dt, h0 * CH:(h0 + 1) * CH],
                                start=(dt == 0),
                                stop=(dt == QH - 1),
                            )
                    for hi in range(2):
                        h0 = hp * 2 + hi
                        ob = outp.tile([P, CH], f32, tag="ob")
                        nc.vector.tensor_copy(out=ob, in_=pss[hi])
                        nc.sync.dma_start(
                            out=out[r0:r0 + P, h0 * CH:(h0 + 1) * CH], in_=ob
                        )

    nc.finalize()
    return nc


_PROGRAMS = {}


def _get_program(S=S_FULL):
    if S not in _PROGRAMS:
        _PROGRAMS[S] = build_program(S)
    return _PROGRAMS[S]


def _host_tables(position_ids_b, S):
    pos = np.asarray(position_ids_b, dtype=np.float32)  # [S]
    inv = 1.0 / (ROPE_THETA ** (np.arange(0, P, 2, dtype=np.float32) / P))  # [64]
    ang = np.concatenate([inv, inv]).astype(np.float32)[:, None] * pos[None, :]
    cosT = np.cos(ang).astype(np.float32)
    sgn = np.where(np.arange(P) < 64, -1.0, 1.0).astype(np.float32)
    sinT = (np.sin(ang) * sgn[:, None]).astype(np.float32)
    return cosT, sinT


def _causal_masks():
    o = np.arange(CH // P)[:, None, None]
    r = np.arange(P)[None, :, None]
    j = np.arange(CH)[None, None, :]
    return ((P * o + r) <= j).astype(BF16)


def make_in_maps(x, position_ids, Wq, Wk, Wv, Wo, Wg, bg, S=S_FULL):
    x = np.asarray(x, dtype=np.float32)
    msk = _causal_masks()
    maps = []
    xT_b = [np.ascontiguousarray(x[b, :S].T).astype(BF16) for b in range(B)]
    tabs = [_host_tables(np.asarray(position_ids)[b, :S], S) for b in range(B)]
    Wq = np.asarray(Wq, np.float32)
    Wk = np.asarray(Wk, np.float32)
    Wv = np.asarray(Wv, np.float32)
    Wo = np.asarray(Wo, np.float32)
    Wg = np.asarray(Wg, np.float32)
    bg = np.asarray(bg, np.float32)
    for core in range(8):
        b, g = core // 4, core % 4
        cosT, sinT = tabs[b]
        maps.append({
            "xT": xT_b[b],
            "wq": np.ascontiguousarray(Wq[:, g * DQ:(g + 1) * DQ]).astype(BF16),
            "wk": np.ascontiguousarray(Wk[:, g * P:(g + 1) * P]).astype(BF16),
            "wv": np.ascontiguousarray(Wv[:, g * P:(g + 1) * P]).astype(BF16),
            "wg": np.ascontiguousarray(Wg[:, g * DQ:(g + 1) * DQ]).astype(BF16),
            "wo": np.ascontiguousarray(Wo[g * DQ:(g + 1) * DQ, :]).astype(BF16),
            "bg": np.ascontiguousarray(bg[g * DQ:(g + 1) * DQ]),
            "cosT": cosT,
            "sinT": sinT,
            "msk": msk,
        })
    return maps


def run(inputs, S=S_FULL, trace=False, **kw):
    nc = _get_program(S)
    maps = make_in_maps(S=S, **inputs)
    res = run_bass_kernel_spmd(nc, maps, core_ids=list(range(8)), trace=trace, **kw)
    out = np.zeros((B, S, HIDDEN), np.float32)
    for core in range(8):
        out[core // 4] += np.asarray(res.results[core]["out"], np.float32)
    return out, res


def kernel(x, position_ids, Wq, Wk, Wv, Wo, Wg, bg):
    out, _ = run(dict(x=x, position_ids=position_ids, Wq=Wq, Wk=Wk, Wv=Wv,
                      Wo=Wo, Wg=Wg, bg=bg))
    return out



# revision 6
# speedup vs baseline: 1.3633x; 1.3633x over previous
"""Trainium2 Bass kernel for LuluAttention (gated GQA attention + RoPE).

Sharding over 8 NeuronCores: core = b*4 + g where b = batch (2), g = head
group (4). Each core computes 4 Q heads + their shared KV head for one batch
element, plus the matching gate slice, and a partial o_proj output
(contraction over its 512 attn dims). Host sums the 4 partials per batch.

Two-pass structure per core:
  Pass A (chunks 0..3): x chunk load -> q/k projections + RoPE -> gate
    (sigmoid) -> v projection. All activations persist in SBUF in transposed
    layout ([dim, seq]) so attention needs no on-chip transposes.
  Pass B (chunks 0..3): causal attention (scoresT = kT.T @ qT per k-tile,
    exp batched 2 tiles per ACTIVATE, triangular-block mask on the diagonal
    128-col block only, attnT accumulated in PSUM), denominator via a dense
    ones-matmul chain over retained prob tiles, reciprocal_approx_fast,
    broadcast via K=1 matmul, gate+normalize muls, then partial o_proj.

This keeps the exp table set (pass B) and sigmoid set (pass A) from
thrashing, keeps TensorE dense (no long PE-idle gaps -> HAM stays at 8/8),
and slices diagonal-tile matmuls to skip the causally-masked column ranges.
"""

import numpy as np
import ml_dtypes
from contextlib import ExitStack

import concourse.bass as bass
import concourse.bacc as bacc
import concourse.tile as tile
from concourse import mybir
from concourse.bass_utils import run_bass_kernel_spmd

BF16 = ml_dtypes.bfloat16

HIDDEN = 2048
B = 2
S_FULL = 2048
P = 128
CH = 512               # seq chunk width
QH = 4                 # q heads per core
DQ = QH * P            # 512 q dims per core
KT = HIDDEN // P       # 16 contraction tiles
SCALE = 1.0 / float(np.sqrt(128.0))
ROPE_THETA = 10000.0


def build_program(S=S_FULL):
    f32 = mybir.dt.float32
    bf16 = mybir.dt.bfloat16
    sig = mybir.ActivationFunctionType.Sigmoid
    expf = mybir.ActivationFunctionType.Exp

    NCH = S // CH
    ST = CH // P           # 4 seq sub-tiles per chunk

    nc = bacc.Bacc("TRN2", debug=False, target_bir_lowering=False)

    xT = nc.declare_dram_parameter("xT", [HIDDEN, S], bf16, False)
    wq = nc.declare_dram_parameter("wq", [HIDDEN, DQ], bf16, False)
    wk = nc.declare_dram_parameter("wk", [HIDDEN, P], bf16, False)
    wv = nc.declare_dram_parameter("wv", [HIDDEN, P], bf16, False)
    wg = nc.declare_dram_parameter("wg", [HIDDEN, DQ], bf16, False)
    wo = nc.declare_dram_parameter("wo", [DQ, HIDDEN], bf16, False)
    bg = nc.declare_dram_parameter("bg", [DQ], f32, False)
    cosT = nc.declare_dram_parameter("cosT", [P, S], bf16, False)
    sinT = nc.declare_dram_parameter("sinT", [P, S], bf16, False)
    msk = nc.declare_dram_parameter("msk", [P, P], bf16, False)
    out = nc.declare_dram_parameter("out", [S, HIDDEN], bf16, True)

    with tile.TileContext(nc) as tc, ExitStack() as ctx:
        wpool = ctx.enter_context(tc.tile_pool(name="weights", bufs=1))
        qkv = ctx.enter_context(tc.tile_pool(name="qkv", bufs=1))

        # ---- persistent loads, ordered so pass-A can start ASAP ----
        wq_sb = wpool.tile([P, KT, DQ], bf16, tag="wq")
        nc.sync.dma_start(out=wq_sb, in_=wq[:, :].rearrange("(kt p) n -> p kt n", p=P))
        wk_sb = wpool.tile([P, KT, P], bf16, tag="wk")
        nc.sync.dma_start(out=wk_sb, in_=wk[:, :].rearrange("(kt p) n -> p kt n", p=P))
        cos_sb = wpool.tile([P, S], bf16, tag="cos")
        nc.sync.dma_start(out=cos_sb, in_=cosT[:, :])
        sin_sb = wpool.tile([P, S], bf16, tag="sin")
        nc.sync.dma_start(out=sin_sb, in_=sinT[:, :])
        wg_sb = wpool.tile([P, KT, DQ], bf16, tag="wg")
        nc.sync.dma_start(out=wg_sb, in_=wg[:, :].rearrange("(kt p) n -> p kt n", p=P))
        bg_sb = wpool.tile([P, QH], f32, tag="bg")
        nc.sync.dma_start(out=bg_sb, in_=bg[:].rearrange("(h p) -> p h", p=P))
        wv_sb = wpool.tile([P, KT, P], bf16, tag="wv")
        nc.sync.dma_start(out=wv_sb, in_=wv[:, :].rearrange("(kt p) n -> p kt n", p=P))
        msk_sb = wpool.tile([P, P], bf16, tag="msk")
        nc.sync.dma_start(out=msk_sb, in_=msk[:, :])
        wo_sb = wpool.tile([P, QH, HIDDEN], bf16, tag="wo")
        nc.sync.dma_start(out=wo_sb, in_=wo[:, :].rearrange("(dt p) n -> p dt n", p=P))
        ones_pv = wpool.tile([P, 1], bf16, tag="ones_pv")
        nc.vector.memset(ones_pv, 1.0)
        ones_bc = wpool.tile([1, P], f32, tag="ones_bc")
        nc.vector.memset(ones_bc, 1.0)

        # persistent per-core activations (transposed layouts)
        qro = qkv.tile([P, QH, S], bf16, tag="qro")
        kro = qkv.tile([P, S], bf16, tag="kro")
        v_sb = qkv.tile([P, S // P, P], bf16, tag="v")
        gt = qkv.tile([P, QH, S], bf16, tag="gt")

        # ================= PASS A: projections =================
        with tc.tile_pool(name="passA", bufs=2) as xpool, \
             tc.tile_pool(name="workA", bufs=3) as work, \
             tc.tile_pool(name="psA", bufs=2, space="PSUM") as psA:
            for c in range(NCH):
                cs = slice(c * CH, (c + 1) * CH)
                xc = xpool.tile([P, KT, CH], bf16, tag="xc")
                nc.sync.dma_start(
                    out=xc, in_=xT[:, cs].rearrange("(kt p) n -> p kt n", p=P)
                )

                # q heads + k, with RoPE applied out of PSUM
                for qh in range(QH + 1):
                    ps = psA.tile([P, CH], f32, tag="proj")
                    for kt in range(KT):
                        lhs = (
                            wq_sb[:, kt, qh * P:(qh + 1) * P]
                            if qh < QH
                            else wk_sb[:, kt, :]
                        )
                        nc.tensor.matmul(
                            ps, lhs, xc[:, kt, :], start=(kt == 0), stop=(kt == KT - 1)
                        )
                    qf = work.tile([P, CH], bf16, tag="qf")
                    nc.scalar.copy(out=qf, in_=ps)
                    rot = work.tile([P, CH], bf16, tag="rot")
                    nc.sync.dma_start(out=rot[0:64, :], in_=qf[64:128, :])
                    nc.sync.dma_start(out=rot[64:128, :], in_=qf[0:64, :])
                    t1 = work.tile([P, CH], bf16, tag="t1")
                    nc.vector.tensor_mul(t1, qf, cos_sb[:, cs])
                    t2 = work.tile([P, CH], bf16, tag="t2")
                    nc.vector.tensor_mul(t2, rot, sin_sb[:, cs])
                    dst = qro[:, qh, cs] if qh < QH else kro[:, cs]
                    nc.vector.tensor_add(dst, t1, t2)

                # gate heads: sigmoid(x @ Wg + bg), transposed layout
                for qh in range(QH):
                    ps = psA.tile([P, CH], f32, tag="proj")
                    for kt in range(KT):
                        nc.tensor.matmul(
                            ps,
                            wg_sb[:, kt, qh * P:(qh + 1) * P],
                            xc[:, kt, :],
                            start=(kt == 0),
                            stop=(kt == KT - 1),
                        )
                    nc.scalar.activation(
                        out=gt[:, qh, cs],
                        in_=ps,
                        func=sig,
                        bias=bg_sb[:, qh:qh + 1],
                        scale=1.0,
                    )

                # v in straight layout [s, d]
                for st in range(ST):
                    s0 = c * ST + st
                    ps = psA.tile([P, P], f32, tag="projv")
                    for kt in range(KT):
                        nc.tensor.matmul(
                            ps,
                            xc[:, kt, st * P:(st + 1) * P],
                            wv_sb[:, kt, :],
                            start=(kt == 0),
                            stop=(kt == KT - 1),
                        )
                    nc.scalar.copy(out=v_sb[:, s0, :], in_=ps)

        # ================= PASS B: attention + o_proj =================
        # PSUM budget (8 banks): sc tag [P,2,CH] x2 bufs = 4 banks (shared by
        # attention score pairs and o_proj output pairs), attn tag x2 = 2,
        # sm tag x2 = 2 (denominator and broadcast cycle the same slots).
        with tc.tile_pool(name="prp", bufs=2) as prp, \
             tc.tile_pool(name="agp", bufs=2) as agp, \
             tc.tile_pool(name="workB", bufs=2) as workB, \
             tc.tile_pool(name="outp", bufs=4) as outp, \
             tc.tile_pool(name="ps_sc", bufs=2, space="PSUM") as ps_sc, \
             tc.tile_pool(name="ps_at", bufs=2, space="PSUM") as ps_at, \
             tc.tile_pool(name="ps_sm", bufs=2, space="PSUM") as ps_sm:
            for c in range(NCH):
                cs = slice(c * CH, (c + 1) * CH)
                ntiles = (c + 1) * ST
                ag = agp.tile([P, QH, CH], bf16, tag="ag")
                for qh in range(QH):
                    at = ps_at.tile([P, CH], f32, tag="attn")
                    pr_all = prp.tile([P, ntiles, CH], bf16, tag="pr", name=f"pr{c}")
                    # scores + exp (batched per tile-pair) + masked av
                    for tp in range((ntiles + 1) // 2):
                        npair = min(2, ntiles - 2 * tp)
                        sc2 = ps_sc.tile([P, 2, CH], f32, tag="sc")
                        for j in range(npair):
                            t = 2 * tp + j
                            o = t - c * ST  # >=0 on diagonal-chunk tiles
                            q0 = o * P if o > 0 else 0
                            nc.tensor.matmul(
                                sc2[:, j, q0:],
                                kro[:, t * P:(t + 1) * P],
                                qro[:, qh, c * CH + q0:(c + 1) * CH],
                                start=True,
                                stop=True,
                            )
                        # exp over both tiles in one ACTIVATE (full width;
                        # causally-dead columns are never read downstream)
                        nc.scalar.activation(
                            out=pr_all[:, 2 * tp:2 * tp + npair, :],
                            in_=sc2[:, 0:npair, :],
                            func=expf,
                            scale=SCALE,
                        )
                        for j in range(npair):
                            t = 2 * tp + j
                            o = t - c * ST
                            if o >= 0:
                                # triangular mask on the diagonal 128-col block
                                nc.vector.tensor_mul(
                                    pr_all[:, t, o * P:(o + 1) * P],
                                    pr_all[:, t, o * P:(o + 1) * P],
                                    msk_sb,
                                )
                        for j in range(npair):
                            t = 2 * tp + j
                            o = t - c * ST
                            q0 = o * P if o > 0 else 0
                            nc.tensor.matmul(
                                at[:, q0:],
                                v_sb[:, t, :],
                                pr_all[:, t, q0:],
                                start=(t == 0),
                                stop=(t == ntiles - 1),
                            )
                    # denominator: dense ones-matmul chain over retained probs
                    dn = ps_sm.tile([P, CH], f32, tag="sm", name="dn")[0:1, :]
                    for t in range(ntiles):
                        o = t - c * ST
                        q0 = o * P if o > 0 else 0
                        nc.tensor.matmul(
                            dn[:, q0:], ones_pv, pr_all[:, t, q0:],
                            start=(t == 0), stop=(t == ntiles - 1),
                        )
                    dn_sb = workB.tile([1, CH], f32, tag="dn_sb")
                    nc.vector.tensor_copy(out=dn_sb, in_=dn)
                    rc = workB.tile([1, CH], f32, tag="recip")
                    nc.vector.reciprocal_approx_fast(out=rc, in_=dn_sb)
                    bc = ps_sm.tile([P, CH], f32, tag="sm", name="bc")
                    nc.tensor.matmul(bc, ones_bc, rc, start=True, stop=True)
                    t3 = workB.tile([P, CH], f32, tag="t3")
                    nc.vector.tensor_mul(t3, at, gt[:, qh, cs])
                    nc.vector.tensor_mul(ag[:, qh, :], t3, bc)

                # ---- partial o_proj for this chunk ----
                for st in range(ST):
                    r0 = c * CH + st * P
                    for hp in range(HIDDEN // CH // 2):
                        pss = ps_sc.tile([P, 2, CH], f32, tag="sc", name="ops")
                        for dt in range(QH):
                            for hi in range(2):
                                h0 = hp * 2 + hi
                                nc.tensor.matmul(
                                    pss[:, hi, :],
                                    ag[:, dt, st * P:(st + 1) * P],
                                    wo_sb[:, dt, h0 * CH:(h0 + 1) * CH],
                                    start=(dt == 0),
                                    stop=(dt == QH - 1),
                                )
                        for hi in range(2):
                            h0 = hp * 2 + hi
                            ob = outp.tile([P, CH], bf16, tag="ob")
                            nc.vector.tensor_copy(out=ob, in_=pss[:, hi, :])
                            nc.sync.dma_start(
                                out=out[r0:r0 + P, h0 * CH:(h0 + 1) * CH], in_=ob
                            )

    nc.finalize()
    return nc


_PROGRAMS = {}


def _get_program(S=S_FULL):
    if S not in _PROGRAMS:
        _PROGRAMS[S] = build_program(S)
    return _PROGRAMS[S]


def _host_tables(position_ids_b, S):
    pos = np.asarray(position_ids_b, dtype=np.float32)  # [S]
    inv = 1.0 / (ROPE_THETA ** (np.arange(0, P, 2, dtype=np.float32) / P))  # [64]
    ang = np.concatenate([inv, inv]).astype(np.float32)[:, None] * pos[None, :]
    cosT = np.cos(ang).astype(BF16)
    sgn = np.where(np.arange(P) < 64, -1.0, 1.0).astype(np.float32)
    sinT = (np.sin(ang) * sgn[:, None]).astype(BF16)
    return cosT, sinT


def _causal_mask():
    r = np.arange(P)[:, None]
    j = np.arange(P)[None, :]
    return (r <= j).astype(BF16)


def make_in_maps(x, position_ids, Wq, Wk, Wv, Wo, Wg, bg, S=S_FULL):
    x = np.asarray(x, dtype=np.float32)
    msk = _causal_mask()
    maps = []
    xT_b = [np.ascontiguousarray(x[b, :S].T).astype(BF16) for b in range(B)]
    tabs = [_host_tables(np.asarray(position_ids)[b, :S], S) for b in range(B)]
    Wq = np.asarray(Wq, np.float32)
    Wk = np.asarray(Wk, np.float32)
    Wv = np.asarray(Wv, np.float32)
    Wo = np.asarray(Wo, np.float32)
    Wg = np.asarray(Wg, np.float32)
    bg = np.asarray(bg, np.float32)
    for core in range(8):
        b, g = core // 4, core % 4
        cosT, sinT = tabs[b]
        maps.append({
            "xT": xT_b[b],
            "wq": np.ascontiguousarray(Wq[:, g * DQ:(g + 1) * DQ]).astype(BF16),
            "wk": np.ascontiguousarray(Wk[:, g * P:(g + 1) * P]).astype(BF16),
            "wv": np.ascontiguousarray(Wv[:, g * P:(g + 1) * P]).astype(BF16),
            "wg": np.ascontiguousarray(Wg[:, g * DQ:(g + 1) * DQ]).astype(BF16),
            "wo": np.ascontiguousarray(Wo[g * DQ:(g + 1) * DQ, :]).astype(BF16),
            "bg": np.ascontiguousarray(bg[g * DQ:(g + 1) * DQ]),
            "cosT": cosT,
            "sinT": sinT,
            "msk": msk,
        })
    return maps


def run(inputs, S=S_FULL, trace=False, **kw):
    nc = _get_program(S)
    maps = make_in_maps(S=S, **inputs)
    res = run_bass_kernel_spmd(nc, maps, core_ids=list(range(8)), trace=trace, **kw)
    out = np.zeros((B, S, HIDDEN), np.float32)
    for core in range(8):
        out[core // 4] += np.asarray(res.results[core]["out"], np.float32)
    return out, res


def kernel(x, position_ids, Wq, Wk, Wv, Wo, Wg, bg):
    out, _ = run(dict(x=x, position_ids=position_ids, Wq=Wq, Wk=Wk, Wv=Wv,
                      Wo=Wo, Wg=Wg, bg=bg))
    return out


# revision 11
# speedup vs baseline: 1.4976x; 1.0985x over previous
"""Trainium2 Bass kernel for LuluAttention (gated GQA attention + RoPE).

Sharding over 8 NeuronCores: core = b*4 + g where b = batch (2), g = head
group (4). Each core computes 4 Q heads + their shared KV head for one batch
element, plus the matching gate slice, and a partial o_proj output
(contraction over its 512 attn dims). Host sums the 4 partials per batch.

Two-pass structure per core:
  Pass A (chunks 0..3): x chunk load -> q/k projections + RoPE -> gate
    (sigmoid) -> v projection. All activations persist in SBUF in transposed
    layout ([dim, seq]) so attention needs no on-chip transposes.
  Pass B (chunks 0..3): causal attention (scoresT = kT.T @ qT per k-tile,
    exp batched 2 tiles per ACTIVATE, triangular-block mask on the diagonal
    128-col block only, attnT accumulated in PSUM), denominator via a dense
    ones-matmul chain over retained prob tiles, reciprocal_approx_fast,
    broadcast via K=1 matmul, gate+normalize muls, then partial o_proj.

This keeps the exp table set (pass B) and sigmoid set (pass A) from
thrashing, keeps TensorE dense (no long PE-idle gaps -> HAM stays at 8/8),
and slices diagonal-tile matmuls to skip the causally-masked column ranges.
"""

import numpy as np
import ml_dtypes
from contextlib import ExitStack

import concourse.bass as bass
import concourse.bacc as bacc
import concourse.tile as tile
from concourse import mybir
from concourse.bass_utils import run_bass_kernel_spmd

BF16 = ml_dtypes.bfloat16

HIDDEN = 2048
B = 2
S_FULL = 2048
P = 128
CH = 512               # seq chunk width
QH = 4                 # q heads per core
DQ = QH * P            # 512 q dims per core
KT = HIDDEN // P       # 16 contraction tiles
SCALE = 1.0 / float(np.sqrt(128.0))
ROPE_THETA = 10000.0


def build_program(S=S_FULL):
    f32 = mybir.dt.float32
    bf16 = mybir.dt.bfloat16
    sig = mybir.ActivationFunctionType.Sigmoid
    expf = mybir.ActivationFunctionType.Exp

    NCH = S // CH
    ST = CH // P           # 4 seq sub-tiles per chunk

    nc = bacc.Bacc("TRN2", debug=False, target_bir_lowering=False)

    xT = nc.declare_dram_parameter("xT", [HIDDEN, S], bf16, False)
    wq = nc.declare_dram_parameter("wq", [HIDDEN, DQ], bf16, False)
    wk = nc.declare_dram_parameter("wk", [HIDDEN, P], bf16, False)
    wv = nc.declare_dram_parameter("wv", [HIDDEN, P], bf16, False)
    wg = nc.declare_dram_parameter("wg", [HIDDEN, DQ], bf16, False)
    wo = nc.declare_dram_parameter("wo", [DQ, HIDDEN], bf16, False)
    bg = nc.declare_dram_parameter("bg", [DQ], f32, False)
    cosT = nc.declare_dram_parameter("cosT", [P, S], bf16, False)
    sinT = nc.declare_dram_parameter("sinT", [P, S], bf16, False)
    msk = nc.declare_dram_parameter("msk", [P, P], bf16, False)
    out = nc.declare_dram_parameter("out", [S, HIDDEN], bf16, True)

    with tile.TileContext(nc) as tc, ExitStack() as ctx:
        wpool = ctx.enter_context(tc.tile_pool(name="weights", bufs=1))
        qkv = ctx.enter_context(tc.tile_pool(name="qkv", bufs=1))

        # ---- persistent tiles; DMAs are issued in ramp-critical order ----
        # (sync-ring DMAs drain FIFO, so the first q-projection's operands
        # must be first in line: wq block 0, then the first x chunk.)
        wq_sb = wpool.tile([P, KT, DQ], bf16, tag="wq")
        nc.sync.dma_start(
            out=wq_sb[:, :, 0:P],
            in_=wq[:, 0:P].rearrange("(kt p) n -> p kt n", p=P),
        )
        wk_sb = wpool.tile([P, KT, P], bf16, tag="wk")
        cos_sb = wpool.tile([P, S], bf16, tag="cos")
        sin_sb = wpool.tile([P, S], bf16, tag="sin")
        wg_sb = wpool.tile([P, KT, DQ], bf16, tag="wg")
        bg_sb = wpool.tile([P, QH], f32, tag="bg")
        wv_sb = wpool.tile([P, KT, P], bf16, tag="wv")
        msk_sb = wpool.tile([P, P], bf16, tag="msk")
        wo_sb = wpool.tile([P, QH, HIDDEN], bf16, tag="wo")
        ones_pv = wpool.tile([P, 1], bf16, tag="ones_pv")
        nc.vector.memset(ones_pv, 1.0)
        ones_bc = wpool.tile([1, P], f32, tag="ones_bc")
        nc.vector.memset(ones_bc, 1.0)

        def load_weights_early():
            for qh in range(1, QH):
                nc.sync.dma_start(
                    out=wq_sb[:, :, qh * P:(qh + 1) * P],
                    in_=wq[:, qh * P:(qh + 1) * P].rearrange(
                        "(kt p) n -> p kt n", p=P),
                )
            nc.sync.dma_start(out=cos_sb, in_=cosT[:, :])
            nc.sync.dma_start(out=sin_sb, in_=sinT[:, :])
            nc.sync.dma_start(
                out=wk_sb, in_=wk[:, :].rearrange("(kt p) n -> p kt n", p=P))
            nc.sync.dma_start(
                out=wv_sb, in_=wv[:, :].rearrange("(kt p) n -> p kt n", p=P))
            nc.sync.dma_start(
                out=wg_sb, in_=wg[:, :].rearrange("(kt p) n -> p kt n", p=P))
            nc.sync.dma_start(out=bg_sb, in_=bg[:].rearrange("(h p) -> p h", p=P))
            nc.sync.dma_start(out=msk_sb, in_=msk[:, :])

        def load_weights_late():
            nc.sync.dma_start(
                out=wo_sb, in_=wo[:, :].rearrange("(dt p) n -> p dt n", p=P))

        # persistent per-core activations (transposed layouts)
        qro = qkv.tile([P, QH, S], bf16, tag="qro")
        kro = qkv.tile([P, S], bf16, tag="kro")
        v_sb = qkv.tile([P, S // P, P], bf16, tag="v")
        gt = qkv.tile([P, QH, S], bf16, tag="gt")

        # ================= PASS A: projections =================
        with tc.tile_pool(name="passA", bufs=2) as xpool, \
             tc.tile_pool(name="workA", bufs=3) as work, \
             tc.tile_pool(name="psA", bufs=2, space="PSUM") as psA:
            for c in range(NCH):
                cs = slice(c * CH, (c + 1) * CH)
                xc = xpool.tile([P, KT, CH], bf16, tag="xc")
                # split halves so the first q chain can start on half 1
                nc.sync.dma_start(
                    out=xc[:, 0:KT // 2, :],
                    in_=xT[0:HIDDEN // 2, cs].rearrange("(kt p) n -> p kt n", p=P),
                )
                nc.sync.dma_start(
                    out=xc[:, KT // 2:, :],
                    in_=xT[HIDDEN // 2:, cs].rearrange("(kt p) n -> p kt n", p=P),
                )
                if c == 0:
                    load_weights_early()
                if c == 1:
                    load_weights_late()

                # q heads + k + v (transposed); RoPE applied to q/k out of PSUM
                for qh in range(QH + 2):
                    ps = psA.tile([P, CH], f32, tag="proj")
                    for kt in range(KT):
                        lhs = (
                            wq_sb[:, kt, qh * P:(qh + 1) * P]
                            if qh < QH
                            else (wk_sb if qh == QH else wv_sb)[:, kt, :]
                        )
                        nc.tensor.matmul(
                            ps, lhs, xc[:, kt, :], start=(kt == 0), stop=(kt == KT - 1)
                        )
                    if qh == QH + 1:
                        # v: cast out of PSUM, then xbar-transpose to [s, d]
                        vt = work.tile([P, CH], bf16, tag="vt")
                        nc.scalar.copy(out=vt, in_=ps)
                        for st in range(ST):
                            nc.sync.dma_start_transpose(
                                out=v_sb[:, c * ST + st, :],
                                in_=vt[:, st * P:(st + 1) * P],
                            )
                        continue
                    qf = work.tile([P, CH], bf16, tag="qf")
                    nc.scalar.copy(out=qf, in_=ps)
                    rot = work.tile([P, CH], bf16, tag="rot")
                    nc.sync.dma_start(out=rot[0:64, :], in_=qf[64:128, :])
                    nc.sync.dma_start(out=rot[64:128, :], in_=qf[0:64, :])
                    t1 = work.tile([P, CH], bf16, tag="t1")
                    nc.vector.tensor_mul(t1, qf, cos_sb[:, cs])
                    t2 = work.tile([P, CH], bf16, tag="t2")
                    nc.vector.tensor_mul(t2, rot, sin_sb[:, cs])
                    dst = qro[:, qh, cs] if qh < QH else kro[:, cs]
                    nc.vector.tensor_add(dst, t1, t2)

                # gate heads: sigmoid(x @ Wg + bg), transposed layout
                for qh in range(QH):
                    ps = psA.tile([P, CH], f32, tag="proj")
                    for kt in range(KT):
                        nc.tensor.matmul(
                            ps,
                            wg_sb[:, kt, qh * P:(qh + 1) * P],
                            xc[:, kt, :],
                            start=(kt == 0),
                            stop=(kt == KT - 1),
                        )
                    nc.scalar.activation(
                        out=gt[:, qh, cs],
                        in_=ps,
                        func=sig,
                        bias=bg_sb[:, qh:qh + 1],
                        scale=1.0,
                    )

        # ================= PASS B: attention + o_proj =================
        # PSUM budget (8 banks): sc tag [P,2,CH] x2 bufs = 4 banks (shared by
        # attention score pairs and o_proj output pairs), attn tag x2 = 2,
        # sm tag x2 = 2 (denominator and broadcast cycle the same slots).
        with tc.tile_pool(name="prp", bufs=2) as prp, \
             tc.tile_pool(name="agp", bufs=2) as agp, \
             tc.tile_pool(name="workB", bufs=2) as workB, \
             tc.tile_pool(name="outp", bufs=4) as outp, \
             tc.tile_pool(name="ps_sc", bufs=2, space="PSUM") as ps_sc, \
             tc.tile_pool(name="ps_at", bufs=2, space="PSUM") as ps_at, \
             tc.tile_pool(name="ps_sm", bufs=2, space="PSUM") as ps_sm:
            for c in range(NCH):
                cs = slice(c * CH, (c + 1) * CH)
                ntiles = (c + 1) * ST
                ag = agp.tile([P, QH, CH], bf16, tag="ag")
                npairs_off = c * ST // 2  # off-diagonal tile pairs per head
                for qh in range(QH):
                    at = ps_at.tile([P, CH], f32, tag="attn")
                    pr_all = prp.tile([P, ntiles, CH], bf16, tag="pr", name=f"pr{c}")
                    pr2 = prp.tile([P, max(npairs_off, 1), CH], bf16, tag="pr2",
                                   name=f"pr2{c}")
                    # scores + exp (batched per tile-pair) + masked av
                    for tp in range((ntiles + 1) // 2):
                        npair = min(2, ntiles - 2 * tp)
                        sc2 = ps_sc.tile([P, 2, CH], f32, tag="sc")
                        for j in range(npair):
                            t = 2 * tp + j
                            o = t - c * ST  # >=0 on diagonal-chunk tiles
                            q0 = o * P if o > 0 else 0
                            nc.tensor.matmul(
                                sc2[:, j, q0:],
                                kro[:, t * P:(t + 1) * P],
                                qro[:, qh, c * CH + q0:(c + 1) * CH],
                                start=True,
                                stop=True,
                            )
                        # exp over both tiles in one ACTIVATE (full width;
                        # causally-dead columns are never read downstream)
                        nc.scalar.activation(
                            out=pr_all[:, 2 * tp:2 * tp + npair, :],
                            in_=sc2[:, 0:npair, :],
                            func=expf,
                            scale=SCALE,
                        )
                        for j in range(npair):
                            t = 2 * tp + j
                            o = t - c * ST
                            if o >= 0:
                                # triangular mask on the diagonal 128-col block
                                nc.vector.tensor_mul(
                                    pr_all[:, t, o * P:(o + 1) * P],
                                    pr_all[:, t, o * P:(o + 1) * P],
                                    msk_sb,
                                )
                        if tp < npairs_off:
                            # pre-sum off-diagonal pairs for the denominator
                            nc.vector.tensor_add(
                                pr2[:, tp, :], pr_all[:, 2 * tp, :],
                                pr_all[:, 2 * tp + 1, :],
                            )
                        for j in range(npair):
                            t = 2 * tp + j
                            o = t - c * ST
                            q0 = o * P if o > 0 else 0
                            nc.tensor.matmul(
                                at[:, q0:],
                                v_sb[:, t, :],
                                pr_all[:, t, q0:],
                                start=(t == 0),
                                stop=(t == ntiles - 1),
                            )
                    # denominator: dense ones-matmul chain (pairs off-diag,
                    # sliced singles on the diagonal chunk)
                    dn = ps_sm.tile([P, CH], f32, tag="sm", name="dn")[0:1, :]
                    n_dn = npairs_off + ST
                    for i in range(n_dn):
                        if i < npairs_off:
                            rhs = pr2[:, i, :]
                        else:
                            o = i - npairs_off
                            q0 = o * P if o > 0 else 0
                            rhs = pr_all[:, c * ST + o, q0:]
                        nc.tensor.matmul(
                            dn[:, CH - rhs.shape[-1]:], ones_pv, rhs,
                            start=(i == 0), stop=(i == n_dn - 1),
                        )
                    dn_sb = workB.tile([1, CH], f32, tag="dn_sb")
                    nc.vector.tensor_copy(out=dn_sb, in_=dn)
                    rc = workB.tile([1, CH], f32, tag="recip")
                    nc.vector.reciprocal_approx_fast(out=rc, in_=dn_sb)
                    bc = ps_sm.tile([P, CH], f32, tag="sm", name="bc")
                    nc.tensor.matmul(bc, ones_bc, rc, start=True, stop=True)
                    t3 = workB.tile([P, CH], f32, tag="t3")
                    nc.vector.tensor_mul(t3, at, gt[:, qh, cs])
                    nc.vector.tensor_mul(ag[:, qh, :], t3, bc)

                # ---- partial o_proj for this chunk ----
                for st in range(ST):
                    r0 = c * CH + st * P
                    for hp in range(HIDDEN // CH // 2):
                        pss = ps_sc.tile([P, 2, CH], f32, tag="sc", name="ops")
                        for dt in range(QH):
                            for hi in range(2):
                                h0 = hp * 2 + hi
                                nc.tensor.matmul(
                                    pss[:, hi, :],
                                    ag[:, dt, st * P:(st + 1) * P],
                                    wo_sb[:, dt, h0 * CH:(h0 + 1) * CH],
                                    start=(dt == 0),
                                    stop=(dt == QH - 1),
                                )
                        ob = outp.tile([P, 2, CH], bf16, tag="ob")
                        nc.vector.tensor_copy(out=ob, in_=pss)
                        nc.sync.dma_start(
                            out=out[r0:r0 + P, hp * 2 * CH:(hp * 2 + 2) * CH],
                            in_=ob,
                        )

    nc.finalize()
    return nc


_PROGRAMS = {}


def _get_program(S=S_FULL):
    if S not in _PROGRAMS:
        _PROGRAMS[S] = build_program(S)
    return _PROGRAMS[S]


def _host_tables(position_ids_b, S):
    pos = np.asarray(position_ids_b, dtype=np.float32)  # [S]
    inv = 1.0 / (ROPE_THETA ** (np.arange(0, P, 2, dtype=np.float32) / P))  # [64]
    ang = np.concatenate([inv, inv]).astype(np.float32)[:, None] * pos[None, :]
    cosT = np.cos(ang).astype(BF16)
    sgn = np.where(np.arange(P) < 64, -1.0, 1.0).astype(np.float32)
    sinT = (np.sin(ang) * sgn[:, None]).astype(BF16)
    return cosT, sinT


def _causal_mask():
    r = np.arange(P)[:, None]
    j = np.arange(P)[None, :]
    return (r <= j).astype(BF16)


def make_in_maps(x, position_ids, Wq, Wk, Wv, Wo, Wg, bg, S=S_FULL):
    x = np.asarray(x, dtype=np.float32)
    msk = _causal_mask()
    maps = []
    xT_b = [np.ascontiguousarray(x[b, :S].T).astype(BF16) for b in range(B)]
    tabs = [_host_tables(np.asarray(position_ids)[b, :S], S) for b in range(B)]
    Wq = np.asarray(Wq, np.float32)
    Wk = np.asarray(Wk, np.float32)
    Wv = np.asarray(Wv, np.float32)
    Wo = np.asarray(Wo, np.float32)
    Wg = np.asarray(Wg, np.float32)
    bg = np.asarray(bg, np.float32)
    for core in range(8):
        b, g = core // 4, core % 4
        cosT, sinT = tabs[b]
        maps.append({
            "xT": xT_b[b],
            "wq": np.ascontiguousarray(Wq[:, g * DQ:(g + 1) * DQ]).astype(BF16),
            "wk": np.ascontiguousarray(Wk[:, g * P:(g + 1) * P]).astype(BF16),
            "wv": np.ascontiguousarray(Wv[:, g * P:(g + 1) * P]).astype(BF16),
            "wg": np.ascontiguousarray(Wg[:, g * DQ:(g + 1) * DQ]).astype(BF16),
            "wo": np.ascontiguousarray(Wo[g * DQ:(g + 1) * DQ, :]).astype(BF16),
            "bg": np.ascontiguousarray(bg[g * DQ:(g + 1) * DQ]),
            "cosT": cosT,
            "sinT": sinT,
            "msk": msk,
        })
    return maps


def run(inputs, S=S_FULL, trace=False, **kw):
    nc = _get_program(S)
    maps = make_in_maps(S=S, **inputs)
    res = run_bass_kernel_spmd(nc, maps, core_ids=list(range(8)), trace=trace, **kw)
    out = np.zeros((B, S, HIDDEN), np.float32)
    for core in range(8):
        out[core // 4] += np.asarray(res.results[core]["out"], np.float32)
    return out, res


def kernel(x, position_ids, Wq, Wk, Wv, Wo, Wg, bg):
    out, _ = run(dict(x=x, position_ids=position_ids, Wq=Wq, Wk=Wk, Wv=Wv,
                      Wo=Wo, Wg=Wg, bg=bg))
    return out


# revision 22
# speedup vs baseline: 1.5161x; 1.0124x over previous
"""Trainium2 Bass kernel for LuluAttention (gated GQA attention + RoPE).

Sharding over 8 NeuronCores: core = b*4 + g where b = batch (2), g = head
group (4). Each core computes 4 Q heads + their shared KV head for one batch
element, plus the matching gate slice, and a partial o_proj output
(contraction over its 512 attn dims). Host sums the 4 partials per batch.

Two-pass structure per core:
  Pass A (chunks 0..3): x chunk load -> q/k projections + RoPE -> gate
    (sigmoid) -> v projection. All activations persist in SBUF in transposed
    layout ([dim, seq]) so attention needs no on-chip transposes.
  Pass B (chunks 0..3): causal attention (scoresT = kT.T @ qT per k-tile,
    exp batched 2 tiles per ACTIVATE, triangular-block mask on the diagonal
    128-col block only, attnT accumulated in PSUM), denominator via a dense
    ones-matmul chain over retained prob tiles, reciprocal_approx_fast,
    broadcast via K=1 matmul, gate+normalize muls, then partial o_proj.

This keeps the exp table set (pass B) and sigmoid set (pass A) from
thrashing, keeps TensorE dense (no long PE-idle gaps -> HAM stays at 8/8),
and slices diagonal-tile matmuls to skip the causally-masked column ranges.
"""

import numpy as np
import ml_dtypes
from contextlib import ExitStack

import concourse.bass as bass
import concourse.bacc as bacc
import concourse.tile as tile
from concourse import mybir
from concourse.bass_utils import run_bass_kernel_spmd

BF16 = ml_dtypes.bfloat16

HIDDEN = 2048
B = 2
S_FULL = 2048
P = 128
CH = 512               # seq chunk width
QH = 4                 # q heads per core
DQ = QH * P            # 512 q dims per core
KT = HIDDEN // P       # 16 contraction tiles
SCALE = 1.0 / float(np.sqrt(128.0))
ROPE_THETA = 10000.0


def build_program(S=S_FULL):
    f32 = mybir.dt.float32
    bf16 = mybir.dt.bfloat16
    tanh = mybir.ActivationFunctionType.Tanh
    expf = mybir.ActivationFunctionType.Exp

    NCH = S // CH
    ST = CH // P           # 4 seq sub-tiles per chunk

    nc = bacc.Bacc("TRN2", debug=False, target_bir_lowering=False)

    xT = nc.declare_dram_parameter("xT", [HIDDEN, S], bf16, False)
    wq = nc.declare_dram_parameter("wq", [HIDDEN, DQ], bf16, False)
    wk = nc.declare_dram_parameter("wk", [HIDDEN, P], bf16, False)
    wv = nc.declare_dram_parameter("wv", [HIDDEN, P], bf16, False)
    wg = nc.declare_dram_parameter("wg", [HIDDEN, DQ], bf16, False)
    wo = nc.declare_dram_parameter("wo", [DQ, HIDDEN], bf16, False)
    bg = nc.declare_dram_parameter("bg", [DQ], f32, False)
    cosT = nc.declare_dram_parameter("cosT", [P, S], bf16, False)
    sinT = nc.declare_dram_parameter("sinT", [P, S], bf16, False)
    msk = nc.declare_dram_parameter("msk", [P, P], bf16, False)
    out = nc.declare_dram_parameter("out", [S, HIDDEN], bf16, True)

    with tile.TileContext(nc) as tc, ExitStack() as ctx:
        wpool = ctx.enter_context(tc.tile_pool(name="weights", bufs=1))
        qkv = ctx.enter_context(tc.tile_pool(name="qkv", bufs=1))

        # ---- persistent tiles; DMAs are issued in ramp-critical order ----
        # (sync-ring DMAs drain FIFO, so the first q-projection's operands
        # must be first in line: wq block 0, then the first x chunk.)
        wq_sb = wpool.tile([P, KT, DQ], bf16, tag="wq")
        for h in range(2):
            nc.sync.dma_start(
                out=wq_sb[:, h * KT // 2:(h + 1) * KT // 2, 0:P],
                in_=wq[h * HIDDEN // 2:(h + 1) * HIDDEN // 2, 0:P].rearrange(
                    "(kt p) n -> p kt n", p=P),
            )
        wk_sb = wpool.tile([P, KT, P], bf16, tag="wk")
        cos_sb = wpool.tile([P, S], bf16, tag="cos")
        sin_sb = wpool.tile([P, S], bf16, tag="sin")
        wg_sb = wpool.tile([P, KT, DQ], bf16, tag="wg")
        bg_sb = wpool.tile([P, QH], f32, tag="bg")
        wv_sb = wpool.tile([P, KT, P], bf16, tag="wv")
        msk_sb = wpool.tile([P, P], bf16, tag="msk")
        wo_sb = wpool.tile([P, QH, HIDDEN], bf16, tag="wo")
        # denominator matmul uses 2.0 so the gate's (1 + tanh)/2 affine is
        # absorbed: ag = at*(1+tanh) * 1/(2*sum(exp))
        ones_pv = wpool.tile([P, 1], bf16, tag="ones_pv")
        nc.vector.memset(ones_pv, 2.0)
        ones_bc = wpool.tile([1, P], f32, tag="ones_bc")
        nc.vector.memset(ones_bc, 1.0)

        # preload the exp/tanh ACT table set during the DMA ramp so no
        # table switch lands mid-kernel
        warm = wpool.tile([1, 2], f32, tag="warm")
        nc.vector.memset(warm, 1.0)
        nc.scalar.activation(out=warm[:, 1:2], in_=warm[:, 0:1], func=tanh)
        nc.scalar.activation(out=warm[:, 0:1], in_=warm[:, 1:2], func=expf)

        def load_weights_early():
            for qh in range(1, QH):
                nc.sync.dma_start(
                    out=wq_sb[:, :, qh * P:(qh + 1) * P],
                    in_=wq[:, qh * P:(qh + 1) * P].rearrange(
                        "(kt p) n -> p kt n", p=P),
                )
            nc.sync.dma_start(out=cos_sb, in_=cosT[:, :])
            nc.sync.dma_start(out=sin_sb, in_=sinT[:, :])
            nc.sync.dma_start(
                out=wk_sb, in_=wk[:, :].rearrange("(kt p) n -> p kt n", p=P))
            nc.sync.dma_start(
                out=wv_sb, in_=wv[:, :].rearrange("(kt p) n -> p kt n", p=P))
            nc.sync.dma_start(
                out=wg_sb, in_=wg[:, :].rearrange("(kt p) n -> p kt n", p=P))
            nc.sync.dma_start(out=bg_sb, in_=bg[:].rearrange("(h p) -> p h", p=P))
            nc.sync.dma_start(out=msk_sb, in_=msk[:, :])

        def load_weights_late():
            nc.sync.dma_start(
                out=wo_sb, in_=wo[:, :].rearrange("(dt p) n -> p dt n", p=P))

        # persistent per-core activations (transposed layouts)
        qro = qkv.tile([P, QH, S], bf16, tag="qro")
        kro = qkv.tile([P, S], bf16, tag="kro")
        v_sb = qkv.tile([P, S // P, P], bf16, tag="v")
        gt = qkv.tile([P, QH, S], bf16, tag="gt")

        # ================= PASS A: projections =================
        with tc.tile_pool(name="passA", bufs=2) as xpool, \
             tc.tile_pool(name="workA", bufs=3) as work, \
             tc.tile_pool(name="psA", bufs=2, space="PSUM") as psA:
            for c in range(NCH):
                cs = slice(c * CH, (c + 1) * CH)
                xc = xpool.tile([P, KT, CH], bf16, tag="xc")
                # split so the first q chain can start on the first piece
                nsp = 4 if c == 0 else 2
                for h in range(nsp):
                    kt0, kt1 = h * KT // nsp, (h + 1) * KT // nsp
                    nc.sync.dma_start(
                        out=xc[:, kt0:kt1, :],
                        in_=xT[kt0 * P:kt1 * P, cs].rearrange(
                            "(kt p) n -> p kt n", p=P),
                    )
                if c == 0:
                    load_weights_early()
                if c == 1:
                    load_weights_late()

                # q heads + k + v (transposed); RoPE applied to q/k out of PSUM
                for qh in range(QH + 2):
                    ps = psA.tile([P, CH], f32, tag="proj")
                    for kt in range(KT):
                        lhs = (
                            wq_sb[:, kt, qh * P:(qh + 1) * P]
                            if qh < QH
                            else (wk_sb if qh == QH else wv_sb)[:, kt, :]
                        )
                        nc.tensor.matmul(
                            ps, lhs, xc[:, kt, :], start=(kt == 0), stop=(kt == KT - 1)
                        )
                    if qh == QH + 1:
                        # v: cast out of PSUM, then xbar-transpose to [s, d]
                        # (scalar HWDGE ring: keeps it off the big-load ring)
                        vt = work.tile([P, CH], bf16, tag="vt")
                        nc.scalar.copy(out=vt, in_=ps)
                        for st in range(ST):
                            nc.scalar.dma_start_transpose(
                                out=v_sb[:, c * ST + st, :],
                                in_=vt[:, st * P:(st + 1) * P],
                            )
                        continue
                    qf = work.tile([P, CH], bf16, tag="qf")
                    nc.scalar.copy(out=qf, in_=ps)
                    # rotate-half via the scalar HWDGE ring: tiny and
                    # latency-critical, must not queue behind weight loads
                    rot = work.tile([P, CH], bf16, tag="rot")
                    nc.scalar.dma_start(out=rot[0:64, :], in_=qf[64:128, :])
                    nc.scalar.dma_start(out=rot[64:128, :], in_=qf[0:64, :])
                    t1 = work.tile([P, CH], bf16, tag="t1")
                    nc.vector.tensor_mul(t1, qf, cos_sb[:, cs])
                    t2 = work.tile([P, CH], bf16, tag="t2")
                    nc.vector.tensor_mul(t2, rot, sin_sb[:, cs])
                    dst = qro[:, qh, cs] if qh < QH else kro[:, cs]
                    nc.vector.tensor_add(dst, t1, t2)

                # gate heads: tanh((x @ Wg + bg)/2), transposed layout.
                # sigmoid = (1+tanh)/2 is finished in pass B's normalize
                # multiply (tanh shares the exp ACT table set; sigmoid won't).
                for qh in range(QH):
                    ps = psA.tile([P, CH], f32, tag="proj")
                    for kt in range(KT):
                        nc.tensor.matmul(
                            ps,
                            wg_sb[:, kt, qh * P:(qh + 1) * P],
                            xc[:, kt, :],
                            start=(kt == 0),
                            stop=(kt == KT - 1),
                        )
                    nc.scalar.activation(
                        out=gt[:, qh, cs],
                        in_=ps,
                        func=tanh,
                        bias=bg_sb[:, qh:qh + 1],
                        scale=0.5,
                    )

        # ================= PASS B: attention + o_proj =================
        # PSUM budget (8 banks): sc tag [P,2,CH] x2 bufs = 4 banks (shared by
        # attention score pairs and o_proj output pairs), attn tag x2 = 2,
        # sm tag x2 = 2 (denominator and broadcast cycle the same slots).
        with tc.tile_pool(name="prp", bufs=2) as prp, \
             tc.tile_pool(name="agp", bufs=2) as agp, \
             tc.tile_pool(name="workB", bufs=2) as workB, \
             tc.tile_pool(name="outp", bufs=4) as outp, \
             tc.tile_pool(name="ps_sc", bufs=2, space="PSUM") as ps_sc, \
             tc.tile_pool(name="ps_at", bufs=2, space="PSUM") as ps_at, \
             tc.tile_pool(name="ps_sm", bufs=2, space="PSUM") as ps_sm:
            def emit_oproj(c, ag):
                # partial o_proj for chunk c; emitted after the next chunk's
                # first attention heads so its ag-dependent matmuls never
                # starve the tensor queue at a chunk boundary
                for st in range(ST):
                    r0 = c * CH + st * P
                    for hp in range(HIDDEN // CH // 2):
                        pss = ps_sc.tile([P, 2, CH], f32, tag="sc", name="ops")
                        for dt in range(QH):
                            for hi in range(2):
                                h0 = hp * 2 + hi
                                nc.tensor.matmul(
                                    pss[:, hi, :],
                                    ag[:, dt, st * P:(st + 1) * P],
                                    wo_sb[:, dt, h0 * CH:(h0 + 1) * CH],
                                    start=(dt == 0),
                                    stop=(dt == QH - 1),
                                )
                        ob = outp.tile([P, 2, CH], bf16, tag="ob")
                        nc.vector.tensor_copy(out=ob, in_=pss)
                        nc.sync.dma_start(
                            out=out[r0:r0 + P, hp * 2 * CH:(hp * 2 + 2) * CH],
                            in_=ob,
                        )

            pending = None
            for c in range(NCH):
                cs = slice(c * CH, (c + 1) * CH)
                ntiles = (c + 1) * ST
                ag = agp.tile([P, QH, CH], bf16, tag="ag")
                npairs_off = c * ST // 2  # off-diagonal tile pairs per head
                for qh in range(QH):
                    at = ps_at.tile([P, CH], f32, tag="attn")
                    pr_all = prp.tile([P, ntiles, CH], bf16, tag="pr", name=f"pr{c}")
                    pr2 = prp.tile([P, max(npairs_off, 1), CH], bf16, tag="pr2",
                                   name=f"pr2{c}")
                    # scores + exp (batched per tile-pair) + masked av
                    for tp in range((ntiles + 1) // 2):
                        npair = min(2, ntiles - 2 * tp)
                        sc2 = ps_sc.tile([P, 2, CH], f32, tag="sc")
                        for j in range(npair):
                            t = 2 * tp + j
                            o = t - c * ST  # >=0 on diagonal-chunk tiles
                            q0 = o * P if o > 0 else 0
                            nc.tensor.matmul(
                                sc2[:, j, q0:],
                                kro[:, t * P:(t + 1) * P],
                                qro[:, qh, c * CH + q0:(c + 1) * CH],
                                start=True,
                                stop=True,
                            )
                        # exp over both tiles in one ACTIVATE (full width;
                        # causally-dead columns are never read downstream)
                        nc.scalar.activation(
                            out=pr_all[:, 2 * tp:2 * tp + npair, :],
                            in_=sc2[:, 0:npair, :],
                            func=expf,
                            scale=SCALE,
                        )
                        for j in range(npair):
                            t = 2 * tp + j
                            o = t - c * ST
                            if o >= 0:
                                # triangular mask on the diagonal 128-col block
                                nc.vector.tensor_mul(
                                    pr_all[:, t, o * P:(o + 1) * P],
                                    pr_all[:, t, o * P:(o + 1) * P],
                                    msk_sb,
                                )
                        if tp < npairs_off:
                            # pre-sum off-diagonal pairs for the denominator
                            nc.vector.tensor_add(
                                pr2[:, tp, :], pr_all[:, 2 * tp, :],
                                pr_all[:, 2 * tp + 1, :],
                            )
                        for j in range(npair):
                            t = 2 * tp + j
                            o = t - c * ST
                            q0 = o * P if o > 0 else 0
                            nc.tensor.matmul(
                                at[:, q0:],
                                v_sb[:, t, :],
                                pr_all[:, t, q0:],
                                start=(t == 0),
                                stop=(t == ntiles - 1),
                            )
                    # denominator: dense ones-matmul chain (pairs off-diag,
                    # sliced singles on the diagonal chunk)
                    dn = ps_sm.tile([P, CH], f32, tag="sm", name="dn")[0:1, :]
                    n_dn = npairs_off + ST
                    for i in range(n_dn):
                        if i < npairs_off:
                            rhs = pr2[:, i, :]
                        else:
                            o = i - npairs_off
                            q0 = o * P if o > 0 else 0
                            rhs = pr_all[:, c * ST + o, q0:]
                        nc.tensor.matmul(
                            dn[:, CH - rhs.shape[-1]:], ones_pv, rhs,
                            start=(i == 0), stop=(i == n_dn - 1),
                        )
                    dn_sb = workB.tile([1, CH], f32, tag="dn_sb")
                    nc.vector.tensor_copy(out=dn_sb, in_=dn)
                    rc = workB.tile([1, CH], f32, tag="recip")
                    nc.vector.reciprocal_approx_fast(out=rc, in_=dn_sb)
                    bc = ps_sm.tile([P, CH], f32, tag="sm", name="bc")
                    nc.tensor.matmul(bc, ones_bc, rc, start=True, stop=True)
                    t3 = workB.tile([P, CH], f32, tag="t3")
                    # t3 = (tanh_gate + 1) * at ; with dn = 2*sum(exp) this
                    # yields ag = at * sigmoid_gate / sum(exp)
                    nc.vector.scalar_tensor_tensor(
                        out=t3, in0=gt[:, qh, cs], scalar=1.0, in1=at,
                        op0=mybir.AluOpType.add, op1=mybir.AluOpType.mult,
                    )
                    nc.vector.tensor_mul(ag[:, qh, :], t3, bc)

                    if qh == 1 and pending is not None:
                        emit_oproj(*pending)
                        pending = None
                pending = (c, ag)
            emit_oproj(*pending)

    nc.finalize()
    return nc


_PROGRAMS = {}


def _get_program(S=S_FULL):
    if S not in _PROGRAMS:
        _PROGRAMS[S] = build_program(S)
    return _PROGRAMS[S]


def _host_tables(position_ids_b, S):
    pos = np.asarray(position_ids_b, dtype=np.float32)  # [S]
    inv = 1.0 / (ROPE_THETA ** (np.arange(0, P, 2, dtype=np.float32) / P))  # [64]
    ang = np.concatenate([inv, inv]).astype(np.float32)[:, None] * pos[None, :]
    cosT = np.cos(ang).astype(BF16)
    sgn = np.where(np.arange(P) < 64, -1.0, 1.0).astype(np.float32)
    sinT = (np.sin(ang) * sgn[:, None]).astype(BF16)
    return cosT, sinT


def _causal_mask():
    r = np.arange(P)[:, None]
    j = np.arange(P)[None, :]
    return (r <= j).astype(BF16)


def make_in_maps(x, position_ids, Wq, Wk, Wv, Wo, Wg, bg, S=S_FULL):
    x = np.asarray(x, dtype=np.float32)
    msk = _causal_mask()
    maps = []
    xT_b = [np.ascontiguousarray(x[b, :S].T).astype(BF16) for b in range(B)]
    tabs = [_host_tables(np.asarray(position_ids)[b, :S], S) for b in range(B)]
    Wq = np.asarray(Wq, np.float32)
    Wk = np.asarray(Wk, np.float32)
    Wv = np.asarray(Wv, np.float32)
    Wo = np.asarray(Wo, np.float32)
    Wg = np.asarray(Wg, np.float32)
    bg = np.asarray(bg, np.float32)
    for core in range(8):
        b, g = core // 4, core % 4
        cosT, sinT = tabs[b]
        maps.append({
            "xT": xT_b[b],
            "wq": np.ascontiguousarray(Wq[:, g * DQ:(g + 1) * DQ]).astype(BF16),
            "wk": np.ascontiguousarray(Wk[:, g * P:(g + 1) * P]).astype(BF16),
            "wv": np.ascontiguousarray(Wv[:, g * P:(g + 1) * P]).astype(BF16),
            "wg": np.ascontiguousarray(Wg[:, g * DQ:(g + 1) * DQ]).astype(BF16),
            "wo": np.ascontiguousarray(Wo[g * DQ:(g + 1) * DQ, :]).astype(BF16),
            "bg": np.ascontiguousarray(0.5 * bg[g * DQ:(g + 1) * DQ]),
            "cosT": cosT,
            "sinT": sinT,
            "msk": msk,
        })
    return maps


def run(inputs, S=S_FULL, trace=False, **kw):
    nc = _get_program(S)
    maps = make_in_maps(S=S, **inputs)
    res = run_bass_kernel_spmd(nc, maps, core_ids=list(range(8)), trace=trace, **kw)
    out = np.zeros((B, S, HIDDEN), np.float32)
    for core in range(8):
        out[core // 4] += np.asarray(res.results[core]["out"], np.float32)
    return out, res


def kernel(x, position_ids, Wq, Wk, Wv, Wo, Wg, bg):
    out, _ = run(dict(x=x, position_ids=position_ids, Wq=Wq, Wk=Wk, Wv=Wv,
                      Wo=Wo, Wg=Wg, bg=bg))
    return out


# revision 25
# speedup vs baseline: 1.7002x; 1.1214x over previous
"""Trainium2 Bass kernel for LuluAttention (gated GQA attention + RoPE).

Sharding over 8 NeuronCores: core = b*4 + g where b = batch (2), g = head
group (4). Each core computes 4 Q heads + their shared KV head for one batch
element, plus the matching gate slice, and a partial o_proj output
(contraction over its 512 attn dims). Host sums the 4 partials per batch.

Two-pass structure per core:
  Pass A (chunks 0..3): x chunk load -> q/k projections + RoPE -> gate
    (sigmoid) -> v projection. All activations persist in SBUF in transposed
    layout ([dim, seq]) so attention needs no on-chip transposes.
  Pass B (chunks 0..3): causal attention (scoresT = kT.T @ qT per k-tile,
    exp batched 2 tiles per ACTIVATE, triangular-block mask on the diagonal
    128-col block only, attnT accumulated in PSUM), denominator via a dense
    ones-matmul chain over retained prob tiles, reciprocal_approx_fast,
    broadcast via K=1 matmul, gate+normalize muls, then partial o_proj.

This keeps the exp table set (pass B) and sigmoid set (pass A) from
thrashing, keeps TensorE dense (no long PE-idle gaps -> HAM stays at 8/8),
and slices diagonal-tile matmuls to skip the causally-masked column ranges.
"""

import numpy as np
import ml_dtypes
from contextlib import ExitStack

import concourse.bass as bass
import concourse.bacc as bacc
import concourse.tile as tile
from concourse import mybir
from concourse.bass_utils import run_bass_kernel_spmd

BF16 = ml_dtypes.bfloat16

HIDDEN = 2048
B = 2
S_FULL = 2048
P = 128
CH = 512               # seq chunk width
QH = 4                 # q heads per core
DQ = QH * P            # 512 q dims per core
KT = HIDDEN // P       # 16 contraction tiles
SCALE = 1.0 / float(np.sqrt(128.0))
ROPE_THETA = 10000.0


def build_program(S=S_FULL):
    f32 = mybir.dt.float32
    bf16 = mybir.dt.bfloat16
    tanh = mybir.ActivationFunctionType.Tanh
    expf = mybir.ActivationFunctionType.Exp

    NCH = S // CH
    ST = CH // P           # 4 seq sub-tiles per chunk

    nc = bacc.Bacc("TRN2", debug=False, target_bir_lowering=False)

    xT = nc.declare_dram_parameter("xT", [HIDDEN, S], bf16, False)
    wq = nc.declare_dram_parameter("wq", [HIDDEN, DQ], bf16, False)
    wk = nc.declare_dram_parameter("wk", [HIDDEN, P], bf16, False)
    wv = nc.declare_dram_parameter("wv", [HIDDEN, P], bf16, False)
    wg = nc.declare_dram_parameter("wg", [HIDDEN, DQ], bf16, False)
    wo = nc.declare_dram_parameter("wo", [DQ, HIDDEN], bf16, False)
    bg = nc.declare_dram_parameter("bg", [DQ], f32, False)
    cosT = nc.declare_dram_parameter("cosT", [P, S], bf16, False)
    sinT = nc.declare_dram_parameter("sinT", [P, S], bf16, False)
    msk = nc.declare_dram_parameter("msk", [P, P], bf16, False)
    out = nc.declare_dram_parameter("out", [S, HIDDEN], bf16, True)

    with tile.TileContext(nc) as tc, ExitStack() as ctx:
        wpool = ctx.enter_context(tc.tile_pool(name="weights", bufs=1))
        qkv = ctx.enter_context(tc.tile_pool(name="qkv", bufs=1))

        # ---- persistent tiles; DMAs are issued in ramp-critical order ----
        # (sync-ring DMAs drain FIFO, so the first q-projection's operands
        # must be first in line: wq block 0, then the first x chunk.)
        wq_sb = wpool.tile([P, KT, DQ], bf16, tag="wq")
        for h in range(2):
            nc.sync.dma_start(
                out=wq_sb[:, h * KT // 2:(h + 1) * KT // 2, 0:P],
                in_=wq[h * HIDDEN // 2:(h + 1) * HIDDEN // 2, 0:P].rearrange(
                    "(kt p) n -> p kt n", p=P),
            )
        wk_sb = wpool.tile([P, KT, P], bf16, tag="wk")
        cos_sb = wpool.tile([P, S], bf16, tag="cos")
        sin_sb = wpool.tile([P, S], bf16, tag="sin")
        wg_sb = wpool.tile([P, KT, DQ], bf16, tag="wg")
        bg_sb = wpool.tile([P, QH], f32, tag="bg")
        wv_sb = wpool.tile([P, KT, P], bf16, tag="wv")
        msk_sb = wpool.tile([P, P], bf16, tag="msk")
        wo_sb = wpool.tile([P, QH, HIDDEN], bf16, tag="wo")
        # denominator matmul: all-2.0 stationary operand broadcasts
        # 2*sum(exp) to every PSUM partition, and the 2x absorbs the gate's
        # (1 + tanh)/2 affine: ag = at*(1+tanh) * 1/(2*sum(exp))
        twos = wpool.tile([P, P], bf16, tag="twos")
        nc.vector.memset(twos, 2.0)

        # preload the exp/tanh ACT table set during the DMA ramp so no
        # table switch lands mid-kernel
        warm = wpool.tile([1, 2], f32, tag="warm")
        nc.vector.memset(warm, 1.0)
        nc.scalar.activation(out=warm[:, 1:2], in_=warm[:, 0:1], func=tanh)
        nc.scalar.activation(out=warm[:, 0:1], in_=warm[:, 1:2], func=expf)

        def load_weights_early():
            for qh in range(1, QH):
                nc.sync.dma_start(
                    out=wq_sb[:, :, qh * P:(qh + 1) * P],
                    in_=wq[:, qh * P:(qh + 1) * P].rearrange(
                        "(kt p) n -> p kt n", p=P),
                )
            nc.sync.dma_start(out=cos_sb, in_=cosT[:, :])
            nc.sync.dma_start(out=sin_sb, in_=sinT[:, :])
            nc.sync.dma_start(
                out=wk_sb, in_=wk[:, :].rearrange("(kt p) n -> p kt n", p=P))
            nc.sync.dma_start(
                out=wv_sb, in_=wv[:, :].rearrange("(kt p) n -> p kt n", p=P))
            nc.sync.dma_start(
                out=wg_sb, in_=wg[:, :].rearrange("(kt p) n -> p kt n", p=P))
            nc.sync.dma_start(out=bg_sb, in_=bg[:].rearrange("(h p) -> p h", p=P))
            nc.sync.dma_start(out=msk_sb, in_=msk[:, :])

        def load_weights_late():
            nc.sync.dma_start(
                out=wo_sb, in_=wo[:, :].rearrange("(dt p) n -> p dt n", p=P))

        # persistent per-core activations (transposed layouts)
        qro = qkv.tile([P, QH, S], bf16, tag="qro")
        kro = qkv.tile([P, S], bf16, tag="kro")
        v_sb = qkv.tile([P, S // P, P], bf16, tag="v")
        gt = qkv.tile([P, QH, S], bf16, tag="gt")

        # ================= PASS A: projections =================
        with tc.tile_pool(name="passA", bufs=2) as xpool, \
             tc.tile_pool(name="workA", bufs=4) as work, \
             tc.tile_pool(name="psA", bufs=4, space="PSUM") as psA:
            for c in range(NCH):
                cs = slice(c * CH, (c + 1) * CH)
                xc = xpool.tile([P, KT, CH], bf16, tag="xc")
                # split so the first q chain can start on the first piece
                nsp = 4 if c == 0 else 2
                for h in range(nsp):
                    kt0, kt1 = h * KT // nsp, (h + 1) * KT // nsp
                    nc.sync.dma_start(
                        out=xc[:, kt0:kt1, :],
                        in_=xT[kt0 * P:kt1 * P, cs].rearrange(
                            "(kt p) n -> p kt n", p=P),
                    )
                if c == 0:
                    load_weights_early()
                if c == 1:
                    load_weights_late()

                # q heads + k + v (transposed); RoPE applied to q/k out of PSUM
                for qh in range(QH + 2):
                    ps = psA.tile([P, CH], f32, tag="proj")
                    for kt in range(KT):
                        lhs = (
                            wq_sb[:, kt, qh * P:(qh + 1) * P]
                            if qh < QH
                            else (wk_sb if qh == QH else wv_sb)[:, kt, :]
                        )
                        nc.tensor.matmul(
                            ps, lhs, xc[:, kt, :], start=(kt == 0), stop=(kt == KT - 1)
                        )
                    if qh == QH + 1:
                        # v: cast out of PSUM, then xbar-transpose to [s, d]
                        # (scalar HWDGE ring: keeps it off the big-load ring)
                        vt = work.tile([P, CH], bf16, tag="vt")
                        nc.scalar.copy(out=vt, in_=ps)
                        for st in range(ST):
                            nc.scalar.dma_start_transpose(
                                out=v_sb[:, c * ST + st, :],
                                in_=vt[:, st * P:(st + 1) * P],
                            )
                        continue
                    qf = work.tile([P, CH], bf16, tag="qf")
                    nc.scalar.copy(out=qf, in_=ps)
                    # rotate-half via the scalar HWDGE ring: tiny and
                    # latency-critical, must not queue behind weight loads
                    rot = work.tile([P, CH], bf16, tag="rot")
                    nc.scalar.dma_start(out=rot[0:64, :], in_=qf[64:128, :])
                    nc.scalar.dma_start(out=rot[64:128, :], in_=qf[0:64, :])
                    t1 = work.tile([P, CH], bf16, tag="t1")
                    nc.vector.tensor_mul(t1, qf, cos_sb[:, cs])
                    t2 = work.tile([P, CH], bf16, tag="t2")
                    nc.vector.tensor_mul(t2, rot, sin_sb[:, cs])
                    dst = qro[:, qh, cs] if qh < QH else kro[:, cs]
                    nc.vector.tensor_add(dst, t1, t2)

                # gate heads: tanh((x @ Wg + bg)/2), transposed layout.
                # sigmoid = (1+tanh)/2 is finished in pass B's normalize
                # multiply (tanh shares the exp ACT table set; sigmoid won't).
                for qh in range(QH):
                    ps = psA.tile([P, CH], f32, tag="proj")
                    for kt in range(KT):
                        nc.tensor.matmul(
                            ps,
                            wg_sb[:, kt, qh * P:(qh + 1) * P],
                            xc[:, kt, :],
                            start=(kt == 0),
                            stop=(kt == KT - 1),
                        )
                    nc.scalar.activation(
                        out=gt[:, qh, cs],
                        in_=ps,
                        func=tanh,
                        bias=bg_sb[:, qh:qh + 1],
                        scale=0.5,
                    )

        # ================= PASS B: attention + o_proj =================
        # PSUM budget (8 banks): sc tag [P,2,CH] x2 bufs = 4 banks (shared by
        # attention score pairs and o_proj output pairs), attn tag x2 = 2,
        # sm tag x2 = 2 (denominator and broadcast cycle the same slots).
        with tc.tile_pool(name="prp", bufs=2) as prp, \
             tc.tile_pool(name="agp", bufs=2) as agp, \
             tc.tile_pool(name="workB", bufs=2) as workB, \
             tc.tile_pool(name="outp", bufs=4) as outp, \
             tc.tile_pool(name="ps_sc", bufs=2, space="PSUM") as ps_sc, \
             tc.tile_pool(name="ps_at", bufs=2, space="PSUM") as ps_at, \
             tc.tile_pool(name="ps_sm", bufs=2, space="PSUM") as ps_sm:
            def emit_oproj(c, ag):
                # partial o_proj for chunk c; emitted after the next chunk's
                # first attention heads so its ag-dependent matmuls never
                # starve the tensor queue at a chunk boundary
                for st in range(ST):
                    r0 = c * CH + st * P
                    for hp in range(HIDDEN // CH // 2):
                        pss = ps_sc.tile([P, 2, CH], f32, tag="sc", name="ops")
                        for dt in range(QH):
                            for hi in range(2):
                                h0 = hp * 2 + hi
                                nc.tensor.matmul(
                                    pss[:, hi, :],
                                    ag[:, dt, st * P:(st + 1) * P],
                                    wo_sb[:, dt, h0 * CH:(h0 + 1) * CH],
                                    start=(dt == 0),
                                    stop=(dt == QH - 1),
                                )
                        ob = outp.tile([P, 2, CH], bf16, tag="ob")
                        nc.vector.tensor_copy(out=ob, in_=pss)
                        nc.sync.dma_start(
                            out=out[r0:r0 + P, hp * 2 * CH:(hp * 2 + 2) * CH],
                            in_=ob,
                        )

            pending = None
            for c in range(NCH):
                cs = slice(c * CH, (c + 1) * CH)
                ntiles = (c + 1) * ST
                ag = agp.tile([P, QH, CH], bf16, tag="ag")
                npairs_off = c * ST // 2  # off-diagonal tile pairs per head
                for qh in range(QH):
                    at = ps_at.tile([P, CH], f32, tag="attn")
                    pr_all = prp.tile([P, ntiles, CH], bf16, tag="pr", name=f"pr{c}")
                    pr2 = prp.tile([P, max(npairs_off, 1), CH], bf16, tag="pr2",
                                   name=f"pr2{c}")
                    # scores + exp (batched per tile-pair) + masked av
                    for tp in range((ntiles + 1) // 2):
                        npair = min(2, ntiles - 2 * tp)
                        sc2 = ps_sc.tile([P, 2, CH], f32, tag="sc")
                        for j in range(npair):
                            t = 2 * tp + j
                            o = t - c * ST  # >=0 on diagonal-chunk tiles
                            q0 = o * P if o > 0 else 0
                            nc.tensor.matmul(
                                sc2[:, j, q0:],
                                kro[:, t * P:(t + 1) * P],
                                qro[:, qh, c * CH + q0:(c + 1) * CH],
                                start=True,
                                stop=True,
                            )
                        # exp over both tiles in one ACTIVATE (full width;
                        # causally-dead columns are never read downstream)
                        nc.scalar.activation(
                            out=pr_all[:, 2 * tp:2 * tp + npair, :],
                            in_=sc2[:, 0:npair, :],
                            func=expf,
                            scale=SCALE,
                        )
                        for j in range(npair):
                            t = 2 * tp + j
                            o = t - c * ST
                            if o >= 0:
                                # triangular mask on the diagonal 128-col block
                                nc.vector.tensor_mul(
                                    pr_all[:, t, o * P:(o + 1) * P],
                                    pr_all[:, t, o * P:(o + 1) * P],
                                    msk_sb,
                                )
                        if tp < npairs_off:
                            # pre-sum off-diagonal pairs for the denominator
                            nc.vector.tensor_add(
                                pr2[:, tp, :], pr_all[:, 2 * tp, :],
                                pr_all[:, 2 * tp + 1, :],
                            )
                        for j in range(npair):
                            t = 2 * tp + j
                            o = t - c * ST
                            q0 = o * P if o > 0 else 0
                            nc.tensor.matmul(
                                at[:, q0:],
                                v_sb[:, t, :],
                                pr_all[:, t, q0:],
                                start=(t == 0),
                                stop=(t == ntiles - 1),
                            )
                    # denominator, broadcast to all partitions by the all-2.0
                    # stationary operand (pairs off-diag, sliced singles on
                    # the diagonal chunk)
                    dn = ps_sm.tile([P, CH], f32, tag="sm", name="dn")
                    n_dn = npairs_off + ST
                    for i in range(n_dn):
                        if i < npairs_off:
                            rhs = pr2[:, i, :]
                        else:
                            o = i - npairs_off
                            q0 = o * P if o > 0 else 0
                            rhs = pr_all[:, c * ST + o, q0:]
                        nc.tensor.matmul(
                            dn[:, CH - rhs.shape[-1]:], twos, rhs,
                            start=(i == 0), stop=(i == n_dn - 1),
                        )
                    rc = workB.tile([P, CH], f32, tag="recip")
                    nc.vector.reciprocal_approx_fast(out=rc, in_=dn)
                    t3 = workB.tile([P, CH], f32, tag="t3")
                    # t3 = (tanh_gate + 1) * at ; with dn = 2*sum(exp) this
                    # yields ag = at * sigmoid_gate / sum(exp)
                    nc.vector.scalar_tensor_tensor(
                        out=t3, in0=gt[:, qh, cs], scalar=1.0, in1=at,
                        op0=mybir.AluOpType.add, op1=mybir.AluOpType.mult,
                    )
                    nc.vector.tensor_mul(ag[:, qh, :], t3, rc)

                    if qh == 1 and pending is not None:
                        emit_oproj(*pending)
                        pending = None
                pending = (c, ag)
            emit_oproj(*pending)

    nc.finalize()
    return nc


_PROGRAMS = {}


def _get_program(S=S_FULL):
    if S not in _PROGRAMS:
        _PROGRAMS[S] = build_program(S)
    return _PROGRAMS[S]


def _host_tables(position_ids_b, S):
    pos = np.asarray(position_ids_b, dtype=np.float32)  # [S]
    inv = 1.0 / (ROPE_THETA ** (np.arange(0, P, 2, dtype=np.float32) / P))  # [64]
    ang = np.concatenate([inv, inv]).astype(np.float32)[:, None] * pos[None, :]
    cosT = np.cos(ang).astype(BF16)
    sgn = np.where(np.arange(P) < 64, -1.0, 1.0).astype(np.float32)
    sinT = (np.sin(ang) * sgn[:, None]).astype(BF16)
    return cosT, sinT


def _causal_mask():
    r = np.arange(P)[:, None]
    j = np.arange(P)[None, :]
    return (r <= j).astype(BF16)


def make_in_maps(x, position_ids, Wq, Wk, Wv, Wo, Wg, bg, S=S_FULL):
    x = np.asarray(x, dtype=np.float32)
    msk = _causal_mask()
    maps = []
    xT_b = [np.ascontiguousarray(x[b, :S].T).astype(BF16) for b in range(B)]
    tabs = [_host_tables(np.asarray(position_ids)[b, :S], S) for b in range(B)]
    Wq = np.asarray(Wq, np.float32)
    Wk = np.asarray(Wk, np.float32)
    Wv = np.asarray(Wv, np.float32)
    Wo = np.asarray(Wo, np.float32)
    Wg = np.asarray(Wg, np.float32)
    bg = np.asarray(bg, np.float32)
    for core in range(8):
        b, g = core // 4, core % 4
        cosT, sinT = tabs[b]
        maps.append({
            "xT": xT_b[b],
            "wq": np.ascontiguousarray(Wq[:, g * DQ:(g + 1) * DQ]).astype(BF16),
            "wk": np.ascontiguousarray(Wk[:, g * P:(g + 1) * P]).astype(BF16),
            "wv": np.ascontiguousarray(Wv[:, g * P:(g + 1) * P]).astype(BF16),
            "wg": np.ascontiguousarray(Wg[:, g * DQ:(g + 1) * DQ]).astype(BF16),
            "wo": np.ascontiguousarray(Wo[g * DQ:(g + 1) * DQ, :]).astype(BF16),
            "bg": np.ascontiguousarray(0.5 * bg[g * DQ:(g + 1) * DQ]),
            "cosT": cosT,
            "sinT": sinT,
            "msk": msk,
        })
    return maps


def run(inputs, S=S_FULL, trace=False, **kw):
    nc = _get_program(S)
    maps = make_in_maps(S=S, **inputs)
    res = run_bass_kernel_spmd(nc, maps, core_ids=list(range(8)), trace=trace, **kw)
    out = np.zeros((B, S, HIDDEN), np.float32)
    for core in range(8):
        out[core // 4] += np.asarray(res.results[core]["out"], np.float32)
    return out, res


def kernel(x, position_ids, Wq, Wk, Wv, Wo, Wg, bg):
    out, _ = run(dict(x=x, position_ids=position_ids, Wq=Wq, Wk=Wk, Wv=Wv,
                      Wo=Wo, Wg=Wg, bg=bg))
    return out


# revision 30
# speedup vs baseline: 1.7011x; 1.0005x over previous
"""Trainium2 Bass kernel for LuluAttention (gated GQA attention + RoPE).

Sharding over 8 NeuronCores: core = b*4 + g where b = batch (2), g = head
group (4). Each core computes 4 Q heads + their shared KV head for one batch
element, plus the matching gate slice, and a partial o_proj output
(contraction over its 512 attn dims). Host sums the 4 partials per batch.

Two-pass structure per core:
  Pass A (chunks 0..3): x chunk load -> q/k projections + RoPE -> gate
    (sigmoid) -> v projection. All activations persist in SBUF in transposed
    layout ([dim, seq]) so attention needs no on-chip transposes.
  Pass B (chunks 0..3): causal attention (scoresT = kT.T @ qT per k-tile,
    exp batched 2 tiles per ACTIVATE, triangular-block mask on the diagonal
    128-col block only, attnT accumulated in PSUM), denominator via a dense
    ones-matmul chain over retained prob tiles, reciprocal_approx_fast,
    broadcast via K=1 matmul, gate+normalize muls, then partial o_proj.

This keeps the exp table set (pass B) and sigmoid set (pass A) from
thrashing, keeps TensorE dense (no long PE-idle gaps -> HAM stays at 8/8),
and slices diagonal-tile matmuls to skip the causally-masked column ranges.
"""

import numpy as np
import ml_dtypes
from contextlib import ExitStack

import concourse.bass as bass
import concourse.bacc as bacc
import concourse.tile as tile
from concourse import mybir
from concourse.bass_utils import run_bass_kernel_spmd

BF16 = ml_dtypes.bfloat16

HIDDEN = 2048
B = 2
S_FULL = 2048
P = 128
CH = 512               # seq chunk width
QH = 4                 # q heads per core
DQ = QH * P            # 512 q dims per core
KT = HIDDEN // P       # 16 contraction tiles
SCALE = 1.0 / float(np.sqrt(128.0))
ROPE_THETA = 10000.0


def build_program(S=S_FULL):
    f32 = mybir.dt.float32
    bf16 = mybir.dt.bfloat16
    tanh = mybir.ActivationFunctionType.Tanh
    expf = mybir.ActivationFunctionType.Exp

    NCH = S // CH
    ST = CH // P           # 4 seq sub-tiles per chunk

    nc = bacc.Bacc("TRN2", debug=False, target_bir_lowering=False)

    xT = nc.declare_dram_parameter("xT", [HIDDEN, S], bf16, False)
    wq = nc.declare_dram_parameter("wq", [HIDDEN, DQ], bf16, False)
    wkv = nc.declare_dram_parameter("wkv", [HIDDEN, 2 * P], bf16, False)
    wg = nc.declare_dram_parameter("wg", [HIDDEN, DQ], bf16, False)
    wo = nc.declare_dram_parameter("wo", [DQ, HIDDEN], bf16, False)
    bg = nc.declare_dram_parameter("bg", [DQ], f32, False)
    cosT = nc.declare_dram_parameter("cosT", [P, S], bf16, False)
    sinT = nc.declare_dram_parameter("sinT", [P, S], bf16, False)
    msk = nc.declare_dram_parameter("msk", [P, P], bf16, False)
    out = nc.declare_dram_parameter("out", [S, HIDDEN], bf16, True)

    with tile.TileContext(nc) as tc, ExitStack() as ctx:
        wpool = ctx.enter_context(tc.tile_pool(name="weights", bufs=1))
        qkv = ctx.enter_context(tc.tile_pool(name="qkv", bufs=1))

        # ---- persistent tiles; DMAs are issued in ramp-critical order ----
        # (sync-ring DMAs drain FIFO, so the first q-projection's operands
        # must be first in line: wq block 0, then the first x chunk.)
        # wq loaded in 4 contraction-row splits (1KB HBM rows, and the first
        # q chain can start as soon as split 0 lands via subtile deps)
        wq_sb = wpool.tile([P, KT, DQ], bf16, tag="wq")
        wkv_sb = wpool.tile([P, KT, 2 * P], bf16, tag="wkv")
        wk_sb = wkv_sb[:, :, 0:P]
        wv_sb = wkv_sb[:, :, P:2 * P]
        cos_sb = wpool.tile([P, S], bf16, tag="cos")
        sin_sb = wpool.tile([P, S], bf16, tag="sin")
        wg_sb = wpool.tile([P, KT, DQ], bf16, tag="wg")
        bg_sb = wpool.tile([P, QH], f32, tag="bg")
        msk_sb = wpool.tile([P, P], bf16, tag="msk")
        wo_sb = wpool.tile([P, QH, HIDDEN], bf16, tag="wo")

        def dma_wq_split(h):
            nc.sync.dma_start(
                out=wq_sb[:, h * KT // 4:(h + 1) * KT // 4, :],
                in_=wq[h * HIDDEN // 4:(h + 1) * HIDDEN // 4, :].rearrange(
                    "(kt p) n -> p kt n", p=P),
            )
        dma_wq_split(0)
        # denominator matmul: all-2.0 stationary operand broadcasts
        # 2*sum(exp) to every PSUM partition, and the 2x absorbs the gate's
        # (1 + tanh)/2 affine: ag = at*(1+tanh) * 1/(2*sum(exp))
        twos = wpool.tile([P, P], bf16, tag="twos")
        nc.vector.memset(twos, 2.0)

        # preload the exp/tanh ACT table set during the DMA ramp so no
        # table switch lands mid-kernel
        warm = wpool.tile([1, 2], f32, tag="warm")
        nc.vector.memset(warm, 1.0)
        nc.scalar.activation(out=warm[:, 1:2], in_=warm[:, 0:1], func=tanh)
        nc.scalar.activation(out=warm[:, 0:1], in_=warm[:, 1:2], func=expf)

        def load_weights_early():
            nc.sync.dma_start(out=cos_sb, in_=cosT[:, :])
            nc.sync.dma_start(out=sin_sb, in_=sinT[:, :])
            nc.sync.dma_start(
                out=wkv_sb, in_=wkv[:, :].rearrange("(kt p) n -> p kt n", p=P))
            nc.sync.dma_start(
                out=wg_sb, in_=wg[:, :].rearrange("(kt p) n -> p kt n", p=P))
            nc.sync.dma_start(out=bg_sb, in_=bg[:].rearrange("(h p) -> p h", p=P))
            nc.sync.dma_start(out=msk_sb, in_=msk[:, :])

        def load_weights_late():
            nc.sync.dma_start(
                out=wo_sb, in_=wo[:, :].rearrange("(dt p) n -> p dt n", p=P))

        # persistent per-core activations (transposed layouts)
        qro = qkv.tile([P, QH, S], bf16, tag="qro")
        kro = qkv.tile([P, S], bf16, tag="kro")
        v_sb = qkv.tile([P, S // P, P], bf16, tag="v")
        gt = qkv.tile([P, QH, S], bf16, tag="gt")

        # ================= PASS A: projections =================
        with tc.tile_pool(name="passA", bufs=2) as xpool, \
             tc.tile_pool(name="workA", bufs=4) as work, \
             tc.tile_pool(name="psA", bufs=4, space="PSUM") as psA:
            for c in range(NCH):
                cs = slice(c * CH, (c + 1) * CH)
                xc = xpool.tile([P, KT, CH], bf16, tag="xc")
                # split so the first q chain can start on the first piece;
                # for chunk 0, interleave with the wq row-splits so both
                # operands stream together
                nsp = 4 if c == 0 else 2
                for h in range(nsp):
                    kt0, kt1 = h * KT // nsp, (h + 1) * KT // nsp
                    nc.sync.dma_start(
                        out=xc[:, kt0:kt1, :],
                        in_=xT[kt0 * P:kt1 * P, cs].rearrange(
                            "(kt p) n -> p kt n", p=P),
                    )
                    if c == 0 and h < 3:
                        dma_wq_split(h + 1)
                if c == 0:
                    load_weights_early()
                if c == 1:
                    load_weights_late()

                # q heads + k + v (transposed); RoPE applied to q/k out of PSUM
                for qh in range(QH + 2):
                    ps = psA.tile([P, CH], f32, tag="proj")
                    for kt in range(KT):
                        lhs = (
                            wq_sb[:, kt, qh * P:(qh + 1) * P]
                            if qh < QH
                            else (wk_sb if qh == QH else wv_sb)[:, kt, :]
                        )
                        nc.tensor.matmul(
                            ps, lhs, xc[:, kt, :], start=(kt == 0), stop=(kt == KT - 1)
                        )
                    if qh == QH + 1:
                        # v: cast out of PSUM, then xbar-transpose to [s, d]
                        # (scalar HWDGE ring: keeps it off the big-load ring)
                        vt = work.tile([P, CH], bf16, tag="vt")
                        nc.scalar.copy(out=vt, in_=ps)
                        for st in range(ST):
                            nc.scalar.dma_start_transpose(
                                out=v_sb[:, c * ST + st, :],
                                in_=vt[:, st * P:(st + 1) * P],
                            )
                        continue
                    qf = work.tile([P, CH], bf16, tag="qf")
                    nc.scalar.copy(out=qf, in_=ps)
                    # rotate-half via the scalar HWDGE ring: tiny and
                    # latency-critical, must not queue behind weight loads
                    rot = work.tile([P, CH], bf16, tag="rot")
                    nc.scalar.dma_start(out=rot[0:64, :], in_=qf[64:128, :])
                    nc.scalar.dma_start(out=rot[64:128, :], in_=qf[0:64, :])
                    t1 = work.tile([P, CH], bf16, tag="t1")
                    nc.vector.tensor_mul(t1, qf, cos_sb[:, cs])
                    t2 = work.tile([P, CH], bf16, tag="t2")
                    nc.vector.tensor_mul(t2, rot, sin_sb[:, cs])
                    dst = qro[:, qh, cs] if qh < QH else kro[:, cs]
                    nc.vector.tensor_add(dst, t1, t2)

                # gate heads: tanh((x @ Wg + bg)/2), transposed layout.
                # sigmoid = (1+tanh)/2 is finished in pass B's normalize
                # multiply (tanh shares the exp ACT table set; sigmoid won't).
                for qh in range(QH):
                    ps = psA.tile([P, CH], f32, tag="proj")
                    for kt in range(KT):
                        nc.tensor.matmul(
                            ps,
                            wg_sb[:, kt, qh * P:(qh + 1) * P],
                            xc[:, kt, :],
                            start=(kt == 0),
                            stop=(kt == KT - 1),
                        )
                    nc.scalar.activation(
                        out=gt[:, qh, cs],
                        in_=ps,
                        func=tanh,
                        bias=bg_sb[:, qh:qh + 1],
                        scale=0.5,
                    )

        # ================= PASS B: attention + o_proj =================
        # PSUM budget (8 banks): sc tag [P,2,CH] x2 bufs = 4 banks (shared by
        # attention score pairs and o_proj output pairs), attn tag x2 = 2,
        # sm tag x2 = 2 (denominator and broadcast cycle the same slots).
        with tc.tile_pool(name="prp", bufs=2) as prp, \
             tc.tile_pool(name="agp", bufs=2) as agp, \
             tc.tile_pool(name="workB", bufs=2) as workB, \
             tc.tile_pool(name="outp", bufs=4) as outp, \
             tc.tile_pool(name="ps_sc", bufs=2, space="PSUM") as ps_sc, \
             tc.tile_pool(name="ps_at", bufs=2, space="PSUM") as ps_at, \
             tc.tile_pool(name="ps_sm", bufs=2, space="PSUM") as ps_sm:
            def emit_oproj(c, ag):
                # partial o_proj for chunk c; emitted after the next chunk's
                # first attention heads so its ag-dependent matmuls never
                # starve the tensor queue at a chunk boundary
                for st in range(ST):
                    r0 = c * CH + st * P
                    for hp in range(HIDDEN // CH // 2):
                        pss = ps_sc.tile([P, 2, CH], f32, tag="sc", name="ops")
                        for dt in range(QH):
                            for hi in range(2):
                                h0 = hp * 2 + hi
                                nc.tensor.matmul(
                                    pss[:, hi, :],
                                    ag[:, dt, st * P:(st + 1) * P],
                                    wo_sb[:, dt, h0 * CH:(h0 + 1) * CH],
                                    start=(dt == 0),
                                    stop=(dt == QH - 1),
                                )
                        ob = outp.tile([P, 2, CH], bf16, tag="ob")
                        nc.vector.tensor_copy(out=ob, in_=pss)
                        nc.sync.dma_start(
                            out=out[r0:r0 + P, hp * 2 * CH:(hp * 2 + 2) * CH],
                            in_=ob,
                        )

            pending = None
            for c in range(NCH):
                cs = slice(c * CH, (c + 1) * CH)
                ntiles = (c + 1) * ST
                ag = agp.tile([P, QH, CH], bf16, tag="ag")
                npairs_off = c * ST // 2  # off-diagonal tile pairs per head
                for qh in range(QH):
                    at = ps_at.tile([P, CH], f32, tag="attn")
                    pr_all = prp.tile([P, ntiles, CH], bf16, tag="pr", name=f"pr{c}")
                    pr2 = prp.tile([P, max(npairs_off, 1), CH], bf16, tag="pr2",
                                   name=f"pr2{c}")
                    # scores + exp (batched per tile-pair) + masked av
                    for tp in range((ntiles + 1) // 2):
                        npair = min(2, ntiles - 2 * tp)
                        sc2 = ps_sc.tile([P, 2, CH], f32, tag="sc")
                        for j in range(npair):
                            t = 2 * tp + j
                            o = t - c * ST  # >=0 on diagonal-chunk tiles
                            q0 = o * P if o > 0 else 0
                            nc.tensor.matmul(
                                sc2[:, j, q0:],
                                kro[:, t * P:(t + 1) * P],
                                qro[:, qh, c * CH + q0:(c + 1) * CH],
                                start=True,
                                stop=True,
                            )
                        # exp over both tiles in one ACTIVATE (full width;
                        # causally-dead columns are never read downstream)
                        nc.scalar.activation(
                            out=pr_all[:, 2 * tp:2 * tp + npair, :],
                            in_=sc2[:, 0:npair, :],
                            func=expf,
                            scale=SCALE,
                        )
                        for j in range(npair):
                            t = 2 * tp + j
                            o = t - c * ST
                            if o >= 0:
                                # triangular mask on the diagonal 128-col block
                                nc.vector.tensor_mul(
                                    pr_all[:, t, o * P:(o + 1) * P],
                                    pr_all[:, t, o * P:(o + 1) * P],
                                    msk_sb,
                                )
                        if tp < npairs_off:
                            # pre-sum off-diagonal pairs for the denominator
                            nc.vector.tensor_add(
                                pr2[:, tp, :], pr_all[:, 2 * tp, :],
                                pr_all[:, 2 * tp + 1, :],
                            )
                        for j in range(npair):
                            t = 2 * tp + j
                            o = t - c * ST
                            q0 = o * P if o > 0 else 0
                            nc.tensor.matmul(
                                at[:, q0:],
                                v_sb[:, t, :],
                                pr_all[:, t, q0:],
                                start=(t == 0),
                                stop=(t == ntiles - 1),
                            )
                    # denominator, broadcast to all partitions by the all-2.0
                    # stationary operand (pairs off-diag, sliced singles on
                    # the diagonal chunk)
                    dn = ps_sm.tile([P, CH], f32, tag="sm", name="dn")
                    n_dn = npairs_off + ST
                    for i in range(n_dn):
                        if i < npairs_off:
                            rhs = pr2[:, i, :]
                        else:
                            o = i - npairs_off
                            q0 = o * P if o > 0 else 0
                            rhs = pr_all[:, c * ST + o, q0:]
                        nc.tensor.matmul(
                            dn[:, CH - rhs.shape[-1]:], twos, rhs,
                            start=(i == 0), stop=(i == n_dn - 1),
                        )
                    rc = workB.tile([P, CH], f32, tag="recip")
                    nc.vector.reciprocal_approx_fast(out=rc, in_=dn)
                    t3 = workB.tile([P, CH], f32, tag="t3")
                    # t3 = (tanh_gate + 1) * at ; with dn = 2*sum(exp) this
                    # yields ag = at * sigmoid_gate / sum(exp)
                    nc.vector.scalar_tensor_tensor(
                        out=t3, in0=gt[:, qh, cs], scalar=1.0, in1=at,
                        op0=mybir.AluOpType.add, op1=mybir.AluOpType.mult,
                    )
                    nc.vector.tensor_mul(ag[:, qh, :], t3, rc)

                    if qh == 1 and pending is not None:
                        emit_oproj(*pending)
                        pending = None
                pending = (c, ag)
            emit_oproj(*pending)

    nc.finalize()
    return nc


_PROGRAMS = {}


def _get_program(S=S_FULL):
    if S not in _PROGRAMS:
        _PROGRAMS[S] = build_program(S)
    return _PROGRAMS[S]


def _host_tables(position_ids_b, S):
    pos = np.asarray(position_ids_b, dtype=np.float32)  # [S]
    inv = 1.0 / (ROPE_THETA ** (np.arange(0, P, 2, dtype=np.float32) / P))  # [64]
    ang = np.concatenate([inv, inv]).astype(np.float32)[:, None] * pos[None, :]
    cosT = np.cos(ang).astype(BF16)
    sgn = np.where(np.arange(P) < 64, -1.0, 1.0).astype(np.float32)
    sinT = (np.sin(ang) * sgn[:, None]).astype(BF16)
    return cosT, sinT


def _causal_mask():
    r = np.arange(P)[:, None]
    j = np.arange(P)[None, :]
    return (r <= j).astype(BF16)


def make_in_maps(x, position_ids, Wq, Wk, Wv, Wo, Wg, bg, S=S_FULL):
    x = np.asarray(x, dtype=np.float32)
    msk = _causal_mask()
    maps = []
    xT_b = [np.ascontiguousarray(x[b, :S].T).astype(BF16) for b in range(B)]
    tabs = [_host_tables(np.asarray(position_ids)[b, :S], S) for b in range(B)]
    Wq = np.asarray(Wq, np.float32)
    Wk = np.asarray(Wk, np.float32)
    Wv = np.asarray(Wv, np.float32)
    Wo = np.asarray(Wo, np.float32)
    Wg = np.asarray(Wg, np.float32)
    bg = np.asarray(bg, np.float32)
    for core in range(8):
        b, g = core // 4, core % 4
        cosT, sinT = tabs[b]
        maps.append({
            "xT": xT_b[b],
            "wq": np.ascontiguousarray(Wq[:, g * DQ:(g + 1) * DQ]).astype(BF16),
            "wkv": np.ascontiguousarray(np.concatenate(
                [Wk[:, g * P:(g + 1) * P], Wv[:, g * P:(g + 1) * P]],
                axis=1)).astype(BF16),
            "wg": np.ascontiguousarray(Wg[:, g * DQ:(g + 1) * DQ]).astype(BF16),
            "wo": np.ascontiguousarray(Wo[g * DQ:(g + 1) * DQ, :]).astype(BF16),
            "bg": np.ascontiguousarray(0.5 * bg[g * DQ:(g + 1) * DQ]),
            "cosT": cosT,
            "sinT": sinT,
            "msk": msk,
        })
    return maps


def run(inputs, S=S_FULL, trace=False, **kw):
    nc = _get_program(S)
    maps = make_in_maps(S=S, **inputs)
    res = run_bass_kernel_spmd(nc, maps, core_ids=list(range(8)), trace=trace, **kw)
    out = np.zeros((B, S, HIDDEN), np.float32)
    for core in range(8):
        out[core // 4] += np.asarray(res.results[core]["out"], np.float32)
    return out, res


def kernel(x, position_ids, Wq, Wk, Wv, Wo, Wg, bg):
    out, _ = run(dict(x=x, position_ids=position_ids, Wq=Wq, Wk=Wk, Wv=Wv,
                      Wo=Wo, Wg=Wg, bg=bg))
    return out
